# revision 6
# baseline (speedup 1.0000x reference)
"""DiT block kernel for 8 Trainium2 NeuronCores (Bass/Tile).

Sharding: sequence dim L=4096 split 8 ways (512 query rows per core).
Each core computes K/V for the full sequence (replicated; no cross-core
collective) but only its own 512 queries through attention, the
out-projection, and the FFN. Inputs are rotated host-side so every
core's local rows sit at positions [0, 512) -> one SPMD program.

Precision: fp8e4 (e4m3) DoubleRow matmuls for all projections, the FFN
and the attention scores (2x TensorE throughput); bf16 for attn @ V;
fp32 accumulation in PSUM; layernorm stats and residuals fp32.
AdaLN modulation is fused into the bf16->fp8 convert of the transposed
LN output. Softmax exp is split between ScalarE (native Exp) and
VectorE (Schraudolph int16-bitcast approximation).
"""

import sys

sys.path.insert(0, "/opt/trn_rl_repo")

from contextlib import ExitStack

import numpy as np
import ml_dtypes

import concourse.bass as bass
import concourse.bacc as bacc
import concourse.tile as tile
import concourse.mybir as mybir
from concourse.bass_utils import run_bass_kernel_spmd

F32 = mybir.dt.float32
BF16 = mybir.dt.bfloat16
FP8 = mybir.dt.float8e4
I16 = mybir.dt.int16
AF = mybir.ActivationFunctionType
OP = mybir.AluOpType
DR = mybir.MatmulPerfMode.DoubleRow

L, D, H, HD, DM = 4096, 768, 12, 64, 3072
NCORES = 8
LQ = L // NCORES  # 512 local query rows
P = 128
EPS = 1e-5
NLC = L // 512  # 8 l-chunks of 512
NKC = L // P  # 32 k-chunks of 128
NQC = LQ // P  # 4 local q-chunks of 128
NDC = D // P  # 6 chunks of the model dim
NHP = H // 2  # 6 head pairs
NMC = DM // P  # 24 chunks of the FFN hidden dim

# host-side power-of-two scales applied to fp8 weights (exact to undo)
SW_QKV = 64.0
SW_AO = 16.0
SW_F1 = 16.0
SW_F2 = 32.0

# Schraudolph exp -> bf16 bits: i16 = round(x*2^7/ln2 + (127*2^7 - C))
SCH_A = 184.6650390625 * 0.125  # includes the hd^-0.5 = 1/8 score scale
SCH_B = 16248.6


def _declare_params(nc):
    dp = nc.declare_dram_parameter
    t = {}
    t["x"] = dp("x", [L, D], F32, isOutput=False)
    t["cond_t"] = dp("cond_t", [P, NDC], F32, isOutput=False)
    t["w_adaln1"] = dp("w_adaln1", [D, 3 * D], BF16, isOutput=False)
    t["w_adaln2"] = dp("w_adaln2", [D, 3 * D], BF16, isOutput=False)
    t["b_adaln1_col"] = dp("b_adaln1_col", [P, 12], F32, isOutput=False)
    t["b_adaln2_col"] = dp("b_adaln2_col", [P, 12], F32, isOutput=False)
    t["b_adaln1_gate"] = dp("b_adaln1_gate", [1, D], F32, isOutput=False)
    t["b_adaln2_gate"] = dp("b_adaln2_gate", [1, D], F32, isOutput=False)
    t["w_qkv8"] = dp("w_qkv8", [D, 3 * D], FP8, isOutput=False)  # * SW_QKV
    t["b_qkv_col"] = dp("b_qkv_col", [P, 18], F32, isOutput=False)
    t["w_ao_bf"] = dp("w_ao_bf", [D, D], BF16, isOutput=False)
    t["b_attn_b"] = dp("b_attn_b", [P, D], F32, isOutput=False)  # b_attn + bv@Wao
    t["w_ffn18"] = dp("w_ffn18", [D, DM], FP8, isOutput=False)  # * SW_F1
    t["b_ffn1_col"] = dp("b_ffn1_col", [P, NMC], F32, isOutput=False)
    t["w_f2_bf"] = dp("w_f2_bf", [DM, D], BF16, isOutput=False)
    t["b_ffn2_b"] = dp("b_ffn2_b", [P, D], F32, isOutput=False)
    t["out"] = dp("out", [LQ, D], F32, isOutput=True)
    return t


def _build_body(nc, tc, ctx, t):
    mm = nc.tensor.matmul
    dma = nc.sync.dma_start
    dmat = nc.sync.dma_start_transpose
    v = nc.vector
    act = nc.scalar.activation

    const = ctx.enter_context(tc.tile_pool(name="const", bufs=1))
    eps_t = const.tile([P, 1], F32)
    v.memset(eps_t, EPS)

    # ---------------- phase A: cond path (SiLU + AdaLN projections) -------
    adaln = ctx.enter_context(tc.tile_pool(name="adaln", bufs=1))
    sc_bf = adaln.tile([P, NDC], BF16)
    sh1_col = adaln.tile([P, NDC], F32)
    sp1_col = adaln.tile([P, NDC], F32)
    sh2_col = adaln.tile([P, NDC], F32)
    sp2_col = adaln.tile([P, NDC], F32)
    g1s_b = adaln.tile([P, D], F32)  # g1 * SW_AO, broadcast
    g2s_b = adaln.tile([P, D], F32)  # g2 * SW_F2, broadcast
    w8ao = adaln.tile([P, NDC, D], FP8)  # w_attn_out * g1 * SW_AO
    xb_bias = adaln.tile([P, D], F32)  # g1 * b_attn_eff
    x2b_bias = adaln.tile([P, D], F32)  # g2 * b_ffn2

    with ExitStack() as phA:
        pool = phA.enter_context(tc.tile_pool(name="phA", bufs=1))
        psA1 = phA.enter_context(tc.tile_pool(name="psA1", bufs=2, space="PSUM"))
        psA2 = phA.enter_context(tc.tile_pool(name="psA2", bufs=2, space="PSUM"))

        cond_sb = pool.tile([P, NDC], F32)
        dma(out=cond_sb[:], in_=t["cond_t"][:])
        sc_f = pool.tile([P, NDC], F32)
        act(sc_f[:], cond_sb[:], AF.Silu)
        v.tensor_copy(sc_bf[:], sc_f[:])

        wa1 = pool.tile([P, NDC, 3 * D], BF16)
        dma(out=wa1[:], in_=t["w_adaln1"].rearrange("(c p) m -> p c m", p=P))
        wa2 = pool.tile([P, NDC, 3 * D], BF16)
        dma(out=wa2[:], in_=t["w_adaln2"].rearrange("(c p) m -> p c m", p=P))
        b1c = pool.tile([P, 12], F32)
        dma(out=b1c[:], in_=t["b_adaln1_col"][:])
        b2c = pool.tile([P, 12], F32)
        dma(out=b2c[:], in_=t["b_adaln2_col"][:])
        b1g = pool.tile([1, D], F32)
        dma(out=b1g[:], in_=t["b_adaln1_gate"][:])
        b2g = pool.tile([1, D], F32)
        dma(out=b2g[:], in_=t["b_adaln2_gate"][:])
        wao_bf = pool.tile([P, NDC, D], BF16)
        dma(out=wao_bf[:], in_=t["w_ao_bf"].rearrange("(c p) m -> p c m", p=P))
        ba_sb = pool.tile([P, D], F32)
        dma(out=ba_sb[:], in_=t["b_attn_b"][:])
        bf2_sb = pool.tile([P, D], F32)
        dma(out=bf2_sb[:], in_=t["b_ffn2_b"][:])

        for r, (wa, bc, bg, sh_col, sp_col, gs_b, sscale) in enumerate(
            [
                (wa1, b1c, b1g, sh1_col, sp1_col, g1s_b, SW_AO),
                (wa2, b2c, b2g, sh2_col, sp2_col, g2s_b, SW_F2),
            ]
        ):
            acol = pool.tile([P, 12], F32, name=f"acol{r}")
            for m in range(12):
                ps = psA1.tile([P, 1], F32)
                for dc in range(NDC):
                    mm(
                        ps[:],
                        wa[:, dc, m * P : (m + 1) * P],
                        sc_bf[:, dc : dc + 1],
                        start=(dc == 0),
                        stop=(dc == NDC - 1),
                    )
                v.tensor_add(acol[:, m : m + 1], ps[:], bc[:, m : m + 1])
            v.tensor_copy(sh_col[:], acol[:, 0:6])
            v.tensor_scalar_add(sp_col[:], acol[:, 6:12], 1.0)
            # gate row, scaled by the weight-quant scale, then broadcast
            g_row = pool.tile([1, D], F32, name=f"grow{r}")
            for j, (n0, n1) in enumerate([(0, 512), (512, 768)]):
                ps = psA2.tile([1, n1 - n0], F32, tag="psg")
                for dc in range(NDC):
                    mm(
                        ps[:],
                        sc_bf[:, dc : dc + 1],
                        wa[:, dc, 2 * D + n0 : 2 * D + n1],
                        start=(dc == 0),
                        stop=(dc == NDC - 1),
                    )
                v.tensor_add(g_row[:, n0:n1], ps[:], bg[:, n0:n1])
            gs_row = pool.tile([1, D], F32, name=f"gsrow{r}")
            v.tensor_scalar_mul(gs_row[:], g_row[:], sscale)
            nc.gpsimd.partition_broadcast(gs_b[:], gs_row[:])

        # fold gates into the fp8 out-proj / ffn2 weights (one quantization)
        for c in range(NDC):
            v.tensor_tensor(w8ao[:, c, :], wao_bf[:, c, :], g1s_b[:], op=OP.mult)
        # bias tiles: xb_bias = (b_attn_eff / SW_AO) * g1s  (per-element)
        v.scalar_tensor_tensor(
            xb_bias[:], ba_sb[:], 1.0 / SW_AO, g1s_b[:], op0=OP.mult, op1=OP.mult
        )
        v.scalar_tensor_tensor(
            x2b_bias[:], bf2_sb[:], 1.0 / SW_F2, g2s_b[:], op0=OP.mult, op1=OP.mult
        )

    # ---------------- phase B: LN1 -> xn1T (fp8, modulated) + QKV ---------
    big = ctx.enter_context(tc.tile_pool(name="big", bufs=1))
    x_loc = big.tile([P, NQC, D], F32)  # local rows for the residual
    x2_loc = [big.tile([P, D], F32, name=f"x2_loc{q}") for q in range(NQC)]
    catT8 = big.tile([P, NDC, LQ], FP8)  # attention output, transposed
    xn2T8 = big.tile([P, NDC, LQ], FP8)
    s_attn = ctx.enter_context(ExitStack())
    attn_pool = s_attn.enter_context(tc.tile_pool(name="attn", bufs=1))
    xn1T8 = [
        attn_pool.tile([P, NDC, 512], FP8, name=f"xn1T8_{i}") for i in range(NLC)
    ]
    v_all = attn_pool.tile([P, NKC, H * (HD + 1)], BF16)  # V + ones column
    # fp8 Q/K in DoubleRow layout: head h -> tile h//3, partitions (h%3)*32
    # (matmul base partition must be 0/32/64, so 3 heads per 128-row tile)
    qT8 = [attn_pool.tile([P, 2, LQ], FP8, name=f"qT8_{i}") for i in range(4)]
    kT8 = [attn_pool.tile([P, 2, L], FP8, name=f"kT8_{i}") for i in range(4)]
    w8qkv = attn_pool.tile([P, NDC, 3 * D], FP8)
    bq_col = attn_pool.tile([P, 18], F32)

    dma(out=w8qkv[:], in_=t["w_qkv8"].rearrange("(c p) m -> p c m", p=P))
    dma(out=bq_col[:], in_=t["b_qkv_col"][:])
    dma(out=x_loc[:], in_=t["x"][0:LQ, :].rearrange("(n p) d -> p n d", p=P))
    v.memset(
        v_all.rearrange("p k (h e) -> p k h e", e=HD + 1)[:, :, :, HD : HD + 1], 1.0
    )

    with ExitStack() as phB:
        xload = phB.enter_context(tc.tile_pool(name="xload", bufs=3))
        spool = phB.enter_context(tc.tile_pool(name="spool", bufs=3))
        nxpool = phB.enter_context(tc.tile_pool(name="nxpool", bufs=3))
        tpool = phB.enter_context(tc.tile_pool(name="tpool", bufs=2))
        ktmp = phB.enter_context(tc.tile_pool(name="ktmp", bufs=3))
        psB1 = phB.enter_context(tc.tile_pool(name="psB1", bufs=2, space="PSUM"))
        psB2 = phB.enter_context(tc.tile_pool(name="psB2", bufs=2, space="PSUM"))

        # LN1 over the full sequence, transposed via DMA-xbar, modulated
        # into fp8 during the convert.
        x_r = t["x"].rearrange("(n p) d -> n p d", p=P)
        xn1T_bf = None
        for i in range(NKC):
            if i % 4 == 0:
                xn1T_bf = tpool.tile([P, NDC, 512], BF16, tag="xnbf")
            xt = xload.tile([P, D], F32)
            dma(out=xt[:], in_=x_r[i])
            stats = spool.tile([P, 2, 6], F32)
            for g in range(2):
                v.bn_stats(stats[:, g, :], xt[:, g * 384 : (g + 1) * 384])
            mv = spool.tile([P, 2], F32)
            v.bn_aggr(mv[:], stats[:])
            sq = spool.tile([P, 1], F32)
            act(sq[:], mv[:, 1:2], AF.Sqrt, bias=eps_t[:, 0:1])
            rstd = spool.tile([P, 1], F32)
            v.reciprocal_approx_fast(rstd[:], sq[:])
            nx = nxpool.tile([P, D], BF16)
            v.tensor_scalar(
                nx[:], xt[:], mv[:, 0:1], rstd[:], op0=OP.subtract, op1=OP.mult
            )
            dmat(out=xn1T_bf[:, :, (i % 4) * P : (i % 4 + 1) * P], in_=nx[:])
            if i % 4 == 3:
                lc = i // 4
                for dc in range(NDC):
                    v.tensor_scalar(
                        xn1T8[lc][:, dc, :],
                        xn1T_bf[:, dc, :],
                        sp1_col[:, dc : dc + 1],
                        sh1_col[:, dc : dc + 1],
                        op0=OP.mult,
                        op1=OP.add,
                    )

        # V for all heads (fp8 DoubleRow), bf16 result (bias folded away)
        v4 = v_all.rearrange("p k (h e) -> p k h e", e=HD + 1)
        for kc in range(NKC):
            ps_v = psB2.tile([P, D], F32)
            for dc2 in range(NDC // 2):
                lhs = xn1T8[kc // 4][:, 2 * dc2 : 2 * dc2 + 2, (kc % 4) * P : (kc % 4 + 1) * P]
                mm(ps_v[:, 0:512], lhs, w8qkv[:, 2 * dc2 : 2 * dc2 + 2, 2 * D : 2 * D + 512],
                   start=(dc2 == 0), stop=(dc2 == NDC // 2 - 1), perf_mode=DR)
                mm(ps_v[:, 512:768], lhs, w8qkv[:, 2 * dc2 : 2 * dc2 + 2, 2 * D + 512 : 3 * D],
                   start=(dc2 == 0), stop=(dc2 == NDC // 2 - 1), perf_mode=DR)
            v.tensor_scalar_mul(
                v4[:, kc, :, 0:HD],
                ps_v.rearrange("p (h e) -> p h e", e=HD),
                1.0 / SW_QKV,
            )

        # Q^T fp8 in DoubleRow layout (local rows only)
        for hp in range(NHP):
            ps_q = psB1.tile([P, 512], F32, tag="mm512")
            for dc2 in range(NDC // 2):
                mm(
                    ps_q[:],
                    w8qkv[:, 2 * dc2 : 2 * dc2 + 2, hp * P : (hp + 1) * P],
                    xn1T8[0][:, 2 * dc2 : 2 * dc2 + 2, :],
                    start=(dc2 == 0),
                    stop=(dc2 == NDC // 2 - 1),
                    perf_mode=DR,
                )
            qtmp = ktmp.tile([P, 512], FP8, tag="qtmp")
            v.tensor_scalar(
                qtmp[:], ps_q[:], 1.0 / SW_QKV, bq_col[:, hp : hp + 1],
                op0=OP.mult, op1=OP.add,
            )
            for dlt in range(2):
                h = 2 * hp + dlt
                for s in range(2):
                    dma(
                        out=qT8[h // 3][(h % 3) * 32 : (h % 3) * 32 + 32, s, :],
                        in_=qtmp[dlt * 64 + s * 32 : dlt * 64 + s * 32 + 32, :],
                    )

        # K^T fp8 in DoubleRow layout, full sequence
        for hp in range(NHP):
            for lc in range(NLC):
                ps_k = psB1.tile([P, 512], F32, tag="mm512")
                for dc2 in range(NDC // 2):
                    mm(
                        ps_k[:],
                        w8qkv[:, 2 * dc2 : 2 * dc2 + 2, D + hp * P : D + (hp + 1) * P],
                        xn1T8[lc][:, 2 * dc2 : 2 * dc2 + 2, :],
                        start=(dc2 == 0),
                        stop=(dc2 == NDC // 2 - 1),
                        perf_mode=DR,
                    )
                ktmp_t = ktmp.tile([P, 512], FP8, tag="ktmp")
                v.tensor_scalar(
                    ktmp_t[:], ps_k[:], 1.0 / SW_QKV, bq_col[:, 6 + hp : 7 + hp],
                    op0=OP.mult, op1=OP.add,
                )
                for dlt in range(2):
                    h = 2 * hp + dlt
                    for s in range(2):
                        dma(
                            out=kT8[h // 3][
                                (h % 3) * 32 : (h % 3) * 32 + 32,
                                s,
                                lc * 512 : (lc + 1) * 512,
                            ],
                            in_=ktmp_t[dlt * 64 + s * 32 : dlt * 64 + s * 32 + 32, :],
                        )

    # ------- phase C: attention pipeline (fp8 DR scores, split exp) --------
    with ExitStack() as phC:
        pt_pool = phC.enter_context(tc.tile_pool(name="ptp", bufs=4))
        tiny = phC.enter_context(tc.tile_pool(name="tiny", bufs=2))
        rzb_pool = phC.enter_context(tc.tile_pool(name="rzb", bufs=2))
        psS = phC.enter_context(tc.tile_pool(name="psS", bufs=2, space="PSUM"))
        psO = phC.enter_context(tc.tile_pool(name="psO", bufs=2, space="PSUM"))

        for h in range(H):
            kt = kT8[h // 3]
            qt = qT8[h // 3]
            p0 = (h % 3) * 32
            ps_o = psO.tile([HD + 1, 512], F32)
            for kc2 in range(NKC // 2):
                ps_s = psS.tile([P, 1024], F32)
                for j in range(2):
                    kc = 2 * kc2 + j
                    mm(
                        ps_s[:, j * 512 : (j + 1) * 512],
                        kt[p0 : p0 + 32, :, kc * P : (kc + 1) * P],
                        qt[p0 : p0 + 32, :, :],
                        start=True,
                        stop=True,
                        perf_mode=DR,
                    )
                ptile = pt_pool.tile([P, 1024], BF16)
                if kc2 % 3 == 2:
                    # Schraudolph exp on VectorE: int16 bits of bf16 e^x
                    v.tensor_scalar(
                        ptile.bitcast(I16)[:], ps_s[:], SCH_A, SCH_B,
                        op0=OP.mult, op1=OP.add,
                    )
                else:
                    act(ptile[:], ps_s[:], AF.Exp, scale=0.125)
                for j in range(2):
                    kc = 2 * kc2 + j
                    mm(
                        ps_o[:],
                        v_all[:, kc, h * (HD + 1) : (h + 1) * (HD + 1)],
                        ptile[:, j * 512 : (j + 1) * 512],
                        start=(kc == 0),
                        stop=(kc == NKC - 1),
                    )
            # normalize columns by the ones-row (softmax denominator)
            zrow = tiny.tile([1, 512], F32)
            v.tensor_copy(zrow[:], ps_o[HD : HD + 1, :])
            rz = tiny.tile([1, 512], F32, tag="rz")
            v.reciprocal_approx_fast(rz[:], zrow[:])
            rz_b = rzb_pool.tile([P, 512], F32)
            nc.gpsimd.partition_broadcast(rz_b[:], rz[:])
            off = (h % 2) * HD
            v.tensor_tensor(
                catT8[off : off + HD, h // 2, :],
                ps_o[0:HD, :],
                rz_b[0:HD, :],
                op=OP.mult,
            )

    s_attn.close()  # free K/V/Q/xn1T space before the FFN weights land

    # -------- phase D: out-projection, residual, LN2 (per-qc fused) ------
    with ExitStack() as phD:
        pool = phD.enter_context(tc.tile_pool(name="phD", bufs=2))
        spool = phD.enter_context(tc.tile_pool(name="spoolE", bufs=3))
        nxpool = phD.enter_context(tc.tile_pool(name="nxE", bufs=2))
        tpool2 = phD.enter_context(tc.tile_pool(name="tpool2", bufs=1))
        psD1 = phD.enter_context(tc.tile_pool(name="psD1", bufs=2, space="PSUM"))
        psD2 = phD.enter_context(tc.tile_pool(name="psD2", bufs=2, space="PSUM"))

        xn2T_bf = tpool2.tile([P, NDC, LQ], BF16)
        for qc in range(NQC):
            ps1 = psD1.tile([P, 512], F32)
            ps2 = psD2.tile([P, 256], F32)
            for cc2 in range(NDC // 2):
                lhs = catT8[:, 2 * cc2 : 2 * cc2 + 2, qc * P : (qc + 1) * P]
                mm(ps1[:], lhs, w8ao[:, 2 * cc2 : 2 * cc2 + 2, 0:512],
                   start=(cc2 == 0), stop=(cc2 == NDC // 2 - 1), perf_mode=DR)
                mm(ps2[:], lhs, w8ao[:, 2 * cc2 : 2 * cc2 + 2, 512:768],
                   start=(cc2 == 0), stop=(cc2 == NDC // 2 - 1), perf_mode=DR)
            # x2 = x + (psum / SW_AO) + g1*b  (bias tile precomputed)
            xq = x2_loc[qc][:]
            v.scalar_tensor_tensor(
                xq[:, 0:512], ps1[:], 1.0 / SW_AO, x_loc[:, qc, 0:512],
                op0=OP.mult, op1=OP.add,
            )
            v.scalar_tensor_tensor(
                xq[:, 512:768], ps2[:], 1.0 / SW_AO, x_loc[:, qc, 512:768],
                op0=OP.mult, op1=OP.add,
            )
            v.tensor_add(xq, xq, xb_bias[:])
            # LN2 for this q-chunk
            stats = spool.tile([P, 2, 6], F32)
            for g in range(2):
                v.bn_stats(stats[:, g, :], xq[:, g * 384 : (g + 1) * 384])
            mv = spool.tile([P, 2], F32)
            v.bn_aggr(mv[:], stats[:])
            sq = spool.tile([P, 1], F32)
            act(sq[:], mv[:, 1:2], AF.Sqrt, bias=eps_t[:, 0:1])
            rstd = spool.tile([P, 1], F32)
            v.reciprocal_approx_fast(rstd[:], sq[:])
            nx = nxpool.tile([P, D], BF16)
            v.tensor_scalar(
                nx[:], xq, mv[:, 0:1], rstd[:], op0=OP.subtract, op1=OP.mult
            )
            dmat(out=xn2T_bf[:, :, qc * P : (qc + 1) * P], in_=nx[:])
        for dc in range(NDC):
            v.tensor_scalar(
                xn2T8[:, dc, :],
                xn2T_bf[:, dc, :],
                sp2_col[:, dc : dc + 1],
                sh2_col[:, dc : dc + 1],
                op0=OP.mult,
                op1=OP.add,
            )

    # ---------------- phase F: FFN + gate + residual -> output -------------
    with ExitStack() as phF:
        wpool = phF.enter_context(tc.tile_pool(name="wffn", bufs=1))
        hpool = phF.enter_context(tc.tile_pool(name="hT", bufs=1))
        pool = phF.enter_context(tc.tile_pool(name="phF", bufs=2))
        psF1 = phF.enter_context(tc.tile_pool(name="psF1", bufs=3, space="PSUM"))
        psF2 = phF.enter_context(tc.tile_pool(name="psF2", bufs=2, space="PSUM"))

        w8f1 = wpool.tile([P, NDC, DM], FP8)
        dma(out=w8f1[:], in_=t["w_ffn18"].rearrange("(c p) m -> p c m", p=P))
        bf1_col = wpool.tile([P, NMC], F32)
        dma(out=bf1_col[:], in_=t["b_ffn1_col"][:])
        wf2_bf = wpool.tile([P, NMC, D], BF16)
        dma(out=wf2_bf[:], in_=t["w_f2_bf"].rearrange("(c p) m -> p c m", p=P))
        w8f2 = wpool.tile([P, NMC, D], FP8)  # w_ffn2 * g2 * SW_F2
        # fold g2 into ffn2 weights -> fp8 (single quantization)
        for c in range(NMC):
            v.tensor_tensor(w8f2[:, c, :], wf2_bf[:, c, :], g2s_b[:], op=OP.mult)

        hT8 = hpool.tile([P, NMC, LQ], FP8)
        for mc in range(NMC):
            ps_h = psF1.tile([P, 512], F32, tag="mm512")
            for dc2 in range(NDC // 2):
                mm(
                    ps_h[:],
                    w8f1[:, 2 * dc2 : 2 * dc2 + 2, mc * P : (mc + 1) * P],
                    xn2T8[:, 2 * dc2 : 2 * dc2 + 2, :],
                    start=(dc2 == 0),
                    stop=(dc2 == NDC // 2 - 1),
                    perf_mode=DR,
                )
            act(
                hT8[:, mc, :], ps_h[:], AF.Gelu,
                bias=bf1_col[:, mc : mc + 1], scale=1.0 / SW_F1,
            )

        out_r = t["out"].rearrange("(n p) d -> n p d", p=P)
        for qc in range(NQC):
            ps1 = psF1.tile([P, 512], F32, tag="mm512")
            ps2 = psF2.tile([P, 256], F32)
            for mc2 in range(NMC // 2):
                lhs = hT8[:, 2 * mc2 : 2 * mc2 + 2, qc * P : (qc + 1) * P]
                mm(ps1[:], lhs, w8f2[:, 2 * mc2 : 2 * mc2 + 2, 0:512],
                   start=(mc2 == 0), stop=(mc2 == NMC // 2 - 1), perf_mode=DR)
                mm(ps2[:], lhs, w8f2[:, 2 * mc2 : 2 * mc2 + 2, 512:768],
                   start=(mc2 == 0), stop=(mc2 == NMC // 2 - 1), perf_mode=DR)
            ot = pool.tile([P, D], F32)
            v.scalar_tensor_tensor(
                ot[:, 0:512], ps1[:], 1.0 / SW_F2, x2_loc[qc][:, 0:512],
                op0=OP.mult, op1=OP.add,
            )
            v.scalar_tensor_tensor(
                ot[:, 512:768], ps2[:], 1.0 / SW_F2, x2_loc[qc][:, 512:768],
                op0=OP.mult, op1=OP.add,
            )
            v.tensor_add(ot[:], ot[:], x2b_bias[:])
            dma(out=out_r[qc], in_=ot[:])


def build_nc():
    nc = bacc.Bacc(None, target_bir_lowering=False, debug=False)
    t = _declare_params(nc)
    with tile.TileContext(nc) as tc:
        with ExitStack() as ctx:
            _build_body(nc, tc, ctx, t)
    nc.compile()
    return nc


_cache = {}


def _prep_in_maps(inputs):
    E4 = ml_dtypes.float8_e4m3fn
    bf = lambda a: np.ascontiguousarray(np.asarray(a, np.float32)).astype(
        ml_dtypes.bfloat16
    )
    f32 = lambda a: np.ascontiguousarray(np.asarray(a, np.float32))
    q8 = lambda a, s: np.ascontiguousarray(
        (np.asarray(a, np.float32) * s).astype(E4)
    )
    x = f32(inputs["x"]).reshape(L, D)
    cond = f32(inputs["cond"]).reshape(D)
    b_adaln1 = f32(inputs["b_adaln1"]).reshape(3 * D)
    b_adaln2 = f32(inputs["b_adaln2"]).reshape(3 * D)
    b_qkv = f32(inputs["b_qkv"]).reshape(3 * D)
    w_ao = f32(inputs["w_attn_out"])
    # fold the V bias through the out-projection: b_attn_eff = b + bv @ Wao
    b_attn_eff = f32(inputs["b_attn_out"]).reshape(D) + b_qkv[2 * D :] @ w_ao
    common = {
        "cond_t": np.ascontiguousarray(cond.reshape(NDC, P).T),
        "w_adaln1": bf(inputs["w_adaln1"]),
        "w_adaln2": bf(inputs["w_adaln2"]),
        "b_adaln1_col": np.ascontiguousarray(b_adaln1[: 12 * P].reshape(12, P).T),
        "b_adaln2_col": np.ascontiguousarray(b_adaln2[: 12 * P].reshape(12, P).T),
        "b_adaln1_gate": np.ascontiguousarray(b_adaln1[2 * D :][None]),
        "b_adaln2_gate": np.ascontiguousarray(b_adaln2[2 * D :][None]),
        "w_qkv8": q8(inputs["w_qkv"], SW_QKV),
        "b_qkv_col": np.ascontiguousarray(b_qkv.reshape(18, P).T),
        "w_ao_bf": bf(w_ao),
        "b_attn_b": np.ascontiguousarray(np.broadcast_to(b_attn_eff, (P, D))),
        "w_ffn18": q8(inputs["w_ffn1"], SW_F1),
        "b_ffn1_col": np.ascontiguousarray(
            f32(inputs["b_ffn1"]).reshape(NMC, P).T
        ),
        "w_f2_bf": bf(inputs["w_ffn2"]),
        "b_ffn2_b": np.ascontiguousarray(
            np.broadcast_to(f32(inputs["b_ffn2"]).reshape(D), (P, D))
        ),
    }
    in_maps = []
    for c in range(NCORES):
        m = dict(common)
        m["x"] = np.ascontiguousarray(np.roll(x, -c * LQ, axis=0))
        in_maps.append(m)
    return in_maps


def kernel(**inputs):
    if "nc" not in _cache:
        _cache["nc"] = build_nc()
    nc = _cache["nc"]
    in_maps = _prep_in_maps(inputs)
    res = run_bass_kernel_spmd(nc, in_maps, list(range(NCORES)))
    out = np.concatenate([res.results[c]["out"] for c in range(NCORES)], axis=0)
    return out.reshape(1, L, D).astype(np.float32)


if __name__ == "__main__":
    rng = np.random.default_rng(0)
    fake = {
        "x": rng.standard_normal((1, L, D), dtype=np.float32),
        "cond": rng.standard_normal((1, D), dtype=np.float32),
        "w_adaln1": rng.standard_normal((D, 3 * D), dtype=np.float32) * 0.02,
        "b_adaln1": np.zeros(3 * D, np.float32),
        "w_qkv": rng.standard_normal((D, 3 * D), dtype=np.float32) * D**-0.5,
        "b_qkv": np.zeros(3 * D, np.float32),
        "w_attn_out": rng.standard_normal((D, D), dtype=np.float32) * D**-0.5,
        "b_attn_out": np.zeros(D, np.float32),
        "w_adaln2": rng.standard_normal((D, 3 * D), dtype=np.float32) * 0.02,
        "b_adaln2": np.zeros(3 * D, np.float32),
        "w_ffn1": rng.standard_normal((D, DM), dtype=np.float32) * D**-0.5,
        "b_ffn1": np.zeros(DM, np.float32),
        "w_ffn2": rng.standard_normal((DM, D), dtype=np.float32) * DM**-0.5,
        "b_ffn2": np.zeros(D, np.float32),
    }
    out = kernel(**fake)
    print("out", out.shape, out.dtype, np.abs(out).max())


# revision 7
# speedup vs baseline: 1.4471x; 1.4471x over previous
"""DiT block kernel for 8 Trainium2 NeuronCores (Bass/Tile).

Sharding: sequence dim L=4096 split 8 ways (512 query rows per core).
Each core computes K/V for the full sequence (replicated; no cross-core
collective) but only its own 512 queries through attention, the
out-projection, and the FFN. Inputs are rotated host-side so every
core's local rows sit at positions [0, 512) -> one SPMD program.

Precision: fp8e4 (e4m3) DoubleRow matmuls for all projections, the FFN
and the attention scores (2x TensorE throughput); bf16 for attn @ V;
fp32 accumulation in PSUM; layernorm stats and residuals fp32.
AdaLN modulation is fused into the bf16->fp8 convert of the transposed
LN output. Softmax exp is split between ScalarE (native Exp) and
VectorE (Schraudolph int16-bitcast approximation).
"""

import sys

sys.path.insert(0, "/opt/trn_rl_repo")

from contextlib import ExitStack

import numpy as np
import ml_dtypes

import concourse.bass as bass
import concourse.bacc as bacc
import concourse.tile as tile
import concourse.mybir as mybir
from concourse.bass_utils import run_bass_kernel_spmd

F32 = mybir.dt.float32
BF16 = mybir.dt.bfloat16
FP8 = mybir.dt.float8e4
I16 = mybir.dt.int16
AF = mybir.ActivationFunctionType
OP = mybir.AluOpType
DR = mybir.MatmulPerfMode.DoubleRow

L, D, H, HD, DM = 4096, 768, 12, 64, 3072
NCORES = 8
LQ = L // NCORES  # 512 local query rows
P = 128
EPS = 1e-5
NLC = L // 512  # 8 l-chunks of 512
NKC = L // P  # 32 k-chunks of 128
NQC = LQ // P  # 4 local q-chunks of 128
NDC = D // P  # 6 chunks of the model dim
NHP = H // 2  # 6 head pairs
NMC = DM // P  # 24 chunks of the FFN hidden dim

# host-side power-of-two scales applied to fp8 weights (exact to undo)
SW_QKV = 64.0
SW_AO = 16.0
SW_F1 = 16.0
SW_F2 = 32.0

# Schraudolph exp -> bf16 bits: i16 = round(x*2^7/ln2 + (127*2^7 - C))
SCH_A = 184.6650390625 * 0.125  # includes the hd^-0.5 = 1/8 score scale
SCH_B = 16248.6


def _declare_params(nc):
    dp = nc.declare_dram_parameter
    t = {}
    t["x"] = dp("x", [L, D], F32, isOutput=False)
    t["cond_t"] = dp("cond_t", [P, NDC], F32, isOutput=False)
    t["w_adaln1"] = dp("w_adaln1", [D, 3 * D], BF16, isOutput=False)
    t["w_adaln2"] = dp("w_adaln2", [D, 3 * D], BF16, isOutput=False)
    t["b_adaln1_col"] = dp("b_adaln1_col", [P, 12], F32, isOutput=False)
    t["b_adaln2_col"] = dp("b_adaln2_col", [P, 12], F32, isOutput=False)
    t["b_adaln1_gate"] = dp("b_adaln1_gate", [1, D], F32, isOutput=False)
    t["b_adaln2_gate"] = dp("b_adaln2_gate", [1, D], F32, isOutput=False)
    t["w_qkv8"] = dp("w_qkv8", [D, 3 * D], FP8, isOutput=False)  # * SW_QKV
    t["b_qkv_col"] = dp("b_qkv_col", [P, 18], F32, isOutput=False)
    t["w_ao_bf"] = dp("w_ao_bf", [D, D], BF16, isOutput=False)
    t["b_attn_b"] = dp("b_attn_b", [P, D], F32, isOutput=False)  # b_attn + bv@Wao
    t["w_ffn18"] = dp("w_ffn18", [D, DM], FP8, isOutput=False)  # * SW_F1
    t["b_ffn1_col"] = dp("b_ffn1_col", [P, NMC], F32, isOutput=False)
    t["w_f2_bf"] = dp("w_f2_bf", [DM, D], BF16, isOutput=False)
    t["b_ffn2_b"] = dp("b_ffn2_b", [P, D], F32, isOutput=False)
    t["out"] = dp("out", [LQ, D], F32, isOutput=True)
    return t


def _build_body(nc, tc, ctx, t):
    mm = nc.tensor.matmul
    dma = nc.sync.dma_start
    dmat = nc.sync.dma_start_transpose
    v = nc.vector
    act = nc.scalar.activation

    const = ctx.enter_context(tc.tile_pool(name="const", bufs=1))
    eps_t = const.tile([P, 1], F32)
    v.memset(eps_t, EPS)

    # ---------------- phase A: cond path (SiLU + AdaLN projections) -------
    adaln = ctx.enter_context(tc.tile_pool(name="adaln", bufs=1))
    sc_bf = adaln.tile([P, NDC], BF16)
    sh1_col = adaln.tile([P, NDC], F32)
    sp1_col = adaln.tile([P, NDC], F32)
    sh2_col = adaln.tile([P, NDC], F32)
    sp2_col = adaln.tile([P, NDC], F32)
    g1s_b = adaln.tile([P, D], F32)  # g1 * SW_AO, broadcast
    g2s_b = adaln.tile([P, D], F32)  # g2 * SW_F2, broadcast
    w8ao = adaln.tile([P, NDC, D], FP8)  # w_attn_out * g1 * SW_AO
    xb_bias = adaln.tile([P, D], F32)  # g1 * b_attn_eff
    x2b_bias = adaln.tile([P, D], F32)  # g2 * b_ffn2

    with ExitStack() as phA:
        pool = phA.enter_context(tc.tile_pool(name="phA", bufs=1))
        psA1 = phA.enter_context(tc.tile_pool(name="psA1", bufs=2, space="PSUM"))
        psA2 = phA.enter_context(tc.tile_pool(name="psA2", bufs=2, space="PSUM"))

        cond_sb = pool.tile([P, NDC], F32)
        dma(out=cond_sb[:], in_=t["cond_t"][:])
        sc_f = pool.tile([P, NDC], F32)
        act(sc_f[:], cond_sb[:], AF.Silu)
        v.tensor_copy(sc_bf[:], sc_f[:])

        wa1 = pool.tile([P, NDC, 3 * D], BF16)
        dma(out=wa1[:], in_=t["w_adaln1"].rearrange("(c p) m -> p c m", p=P))
        wa2 = pool.tile([P, NDC, 3 * D], BF16)
        dma(out=wa2[:], in_=t["w_adaln2"].rearrange("(c p) m -> p c m", p=P))
        b1c = pool.tile([P, 12], F32)
        dma(out=b1c[:], in_=t["b_adaln1_col"][:])
        b2c = pool.tile([P, 12], F32)
        dma(out=b2c[:], in_=t["b_adaln2_col"][:])
        b1g = pool.tile([1, D], F32)
        dma(out=b1g[:], in_=t["b_adaln1_gate"][:])
        b2g = pool.tile([1, D], F32)
        dma(out=b2g[:], in_=t["b_adaln2_gate"][:])
        wao_bf = pool.tile([P, NDC, D], BF16)
        dma(out=wao_bf[:], in_=t["w_ao_bf"].rearrange("(c p) m -> p c m", p=P))
        ba_sb = pool.tile([P, D], F32)
        dma(out=ba_sb[:], in_=t["b_attn_b"][:])
        bf2_sb = pool.tile([P, D], F32)
        dma(out=bf2_sb[:], in_=t["b_ffn2_b"][:])

        for r, (wa, bc, bg, sh_col, sp_col, gs_b, sscale) in enumerate(
            [
                (wa1, b1c, b1g, sh1_col, sp1_col, g1s_b, SW_AO),
                (wa2, b2c, b2g, sh2_col, sp2_col, g2s_b, SW_F2),
            ]
        ):
            acol = pool.tile([P, 12], F32, name=f"acol{r}")
            for m in range(12):
                ps = psA1.tile([P, 1], F32)
                for dc in range(NDC):
                    mm(
                        ps[:],
                        wa[:, dc, m * P : (m + 1) * P],
                        sc_bf[:, dc : dc + 1],
                        start=(dc == 0),
                        stop=(dc == NDC - 1),
                    )
                v.tensor_add(acol[:, m : m + 1], ps[:], bc[:, m : m + 1])
            v.tensor_copy(sh_col[:], acol[:, 0:6])
            v.tensor_scalar_add(sp_col[:], acol[:, 6:12], 1.0)
            # gate row, scaled by the weight-quant scale, then broadcast
            g_row = pool.tile([1, D], F32, name=f"grow{r}")
            for j, (n0, n1) in enumerate([(0, 512), (512, 768)]):
                ps = psA2.tile([1, n1 - n0], F32, tag="psg")
                for dc in range(NDC):
                    mm(
                        ps[:],
                        sc_bf[:, dc : dc + 1],
                        wa[:, dc, 2 * D + n0 : 2 * D + n1],
                        start=(dc == 0),
                        stop=(dc == NDC - 1),
                    )
                v.tensor_add(g_row[:, n0:n1], ps[:], bg[:, n0:n1])
            gs_row = pool.tile([1, D], F32, name=f"gsrow{r}")
            v.tensor_scalar_mul(gs_row[:], g_row[:], sscale)
            nc.gpsimd.partition_broadcast(gs_b[:], gs_row[:])

        # fold gates into the fp8 out-proj / ffn2 weights (one quantization)
        for c in range(NDC):
            v.tensor_tensor(w8ao[:, c, :], wao_bf[:, c, :], g1s_b[:], op=OP.mult)
        # bias tiles: xb_bias = (b_attn_eff / SW_AO) * g1s  (per-element)
        v.scalar_tensor_tensor(
            xb_bias[:], ba_sb[:], 1.0 / SW_AO, g1s_b[:], op0=OP.mult, op1=OP.mult
        )
        v.scalar_tensor_tensor(
            x2b_bias[:], bf2_sb[:], 1.0 / SW_F2, g2s_b[:], op0=OP.mult, op1=OP.mult
        )

    # ---------------- phase B: LN1 -> xn1T (fp8, modulated) + V/Q ---------
    big = ctx.enter_context(tc.tile_pool(name="big", bufs=1))
    x_loc = big.tile([P, NQC, D], F32)  # local rows for the residual
    x2_loc = [big.tile([P, D], F32, name=f"x2_loc{q}") for q in range(NQC)]
    catT8 = big.tile([P, NDC, LQ], FP8)  # attention output, transposed
    xn2T8 = big.tile([P, NDC, LQ], FP8)
    s_attn = ctx.enter_context(ExitStack())
    attn_pool = s_attn.enter_context(tc.tile_pool(name="attn", bufs=1))
    xn1T8 = [
        attn_pool.tile([P, NDC, 512], FP8, name=f"xn1T8_{i}") for i in range(NLC)
    ]
    v_all = attn_pool.tile([P, NKC, H * (HD + 1)], BF16)  # V + ones column
    qT_all = attn_pool.tile([P, NHP, LQ], BF16)
    w8qkv = attn_pool.tile([P, NDC, 3 * D], FP8)
    bq_col = attn_pool.tile([P, 18], F32)

    dma(out=w8qkv[:], in_=t["w_qkv8"].rearrange("(c p) m -> p c m", p=P))
    dma(out=bq_col[:], in_=t["b_qkv_col"][:])
    dma(out=x_loc[:], in_=t["x"][0:LQ, :].rearrange("(n p) d -> p n d", p=P))
    v.memset(
        v_all.rearrange("p k (h e) -> p k h e", e=HD + 1)[:, :, :, HD : HD + 1], 1.0
    )

    with ExitStack() as phB:
        xload = phB.enter_context(tc.tile_pool(name="xload", bufs=3))
        spool = phB.enter_context(tc.tile_pool(name="spool", bufs=3))
        nxpool = phB.enter_context(tc.tile_pool(name="nxpool", bufs=3))
        tpool = phB.enter_context(tc.tile_pool(name="tpool", bufs=2))
        psB1 = phB.enter_context(tc.tile_pool(name="psB1", bufs=2, space="PSUM"))
        psB2 = phB.enter_context(tc.tile_pool(name="psB2", bufs=2, space="PSUM"))

        # LN1 over the full sequence, transposed via DMA-xbar, modulated
        # into fp8 during the convert.
        x_r = t["x"].rearrange("(n p) d -> n p d", p=P)
        xn1T_bf = None
        for i in range(NKC):
            if i % 4 == 0:
                xn1T_bf = tpool.tile([P, NDC, 512], BF16, tag="xnbf")
            xt = xload.tile([P, D], F32)
            dma(out=xt[:], in_=x_r[i])
            stats = spool.tile([P, 2, 6], F32)
            for g in range(2):
                v.bn_stats(stats[:, g, :], xt[:, g * 384 : (g + 1) * 384])
            mv = spool.tile([P, 2], F32)
            v.bn_aggr(mv[:], stats[:])
            sq = spool.tile([P, 1], F32)
            act(sq[:], mv[:, 1:2], AF.Sqrt, bias=eps_t[:, 0:1])
            rstd = spool.tile([P, 1], F32)
            v.reciprocal_approx_fast(rstd[:], sq[:])
            nmr = spool.tile([P, 1], F32)
            v.scalar_tensor_tensor(
                nmr[:], mv[:, 0:1], -1.0, rstd[:], op0=OP.mult, op1=OP.mult
            )
            nx = nxpool.tile([P, D], BF16)
            act(nx[:], xt[:], AF.Identity, bias=nmr[:, 0:1], scale=rstd[:, 0:1])
            dmat(out=xn1T_bf[:, :, (i % 4) * P : (i % 4 + 1) * P], in_=nx[:])
            if i % 4 == 3:
                lc = i // 4
                for dc in range(NDC):
                    v.tensor_scalar(
                        xn1T8[lc][:, dc, :],
                        xn1T_bf[:, dc, :],
                        sp1_col[:, dc : dc + 1],
                        sh1_col[:, dc : dc + 1],
                        op0=OP.mult,
                        op1=OP.add,
                    )

        # V for all heads (fp8 DoubleRow), bf16 result (bias folded away)
        v4 = v_all.rearrange("p k (h e) -> p k h e", e=HD + 1)
        for kc in range(NKC):
            ps_v = psB2.tile([P, D], F32)
            for dc2 in range(NDC // 2):
                lhs = xn1T8[kc // 4][:, 2 * dc2 : 2 * dc2 + 2, (kc % 4) * P : (kc % 4 + 1) * P]
                mm(ps_v[:, 0:512], lhs, w8qkv[:, 2 * dc2 : 2 * dc2 + 2, 2 * D : 2 * D + 512],
                   start=(dc2 == 0), stop=(dc2 == NDC // 2 - 1), perf_mode=DR)
                mm(ps_v[:, 512:768], lhs, w8qkv[:, 2 * dc2 : 2 * dc2 + 2, 2 * D + 512 : 3 * D],
                   start=(dc2 == 0), stop=(dc2 == NDC // 2 - 1), perf_mode=DR)
            act(
                v4[:, kc, :, 0:HD],
                ps_v.rearrange("p (h e) -> p h e", e=HD),
                AF.Copy,
                scale=1.0 / SW_QKV,
            )

        # Q^T bf16 (local rows only)
        for hp in range(NHP):
            ps_q = psB1.tile([P, 512], F32, tag="mm512")
            for dc2 in range(NDC // 2):
                mm(
                    ps_q[:],
                    w8qkv[:, 2 * dc2 : 2 * dc2 + 2, hp * P : (hp + 1) * P],
                    xn1T8[0][:, 2 * dc2 : 2 * dc2 + 2, :],
                    start=(dc2 == 0),
                    stop=(dc2 == NDC // 2 - 1),
                    perf_mode=DR,
                )
            v.tensor_scalar(
                qT_all[:, hp, :], ps_q[:], 1.0 / SW_QKV, bq_col[:, hp : hp + 1],
                op0=OP.mult, op1=OP.add,
            )

    # ------- phase C: merged K-projection + attention pipeline -------------
    with ExitStack() as phC:
        kv_pool = phC.enter_context(tc.tile_pool(name="kvp", bufs=2))
        pt_pool = phC.enter_context(tc.tile_pool(name="ptp", bufs=4))
        tiny = phC.enter_context(tc.tile_pool(name="tiny", bufs=2))
        rzb_pool = phC.enter_context(tc.tile_pool(name="rzb", bufs=2))
        psS = phC.enter_context(tc.tile_pool(name="psS", bufs=2, space="PSUM"))
        psO = phC.enter_context(tc.tile_pool(name="psO", bufs=2, space="PSUM"))
        psK = phC.enter_context(tc.tile_pool(name="psK", bufs=2, space="PSUM"))

        for hp in range(NHP):
            # K^T for this head pair (overlaps with attention on hp-1)
            kT = kv_pool.tile([P, L], BF16, tag="kT")
            for lc in range(NLC):
                ps_k = psK.tile([P, 512], F32)
                for dc2 in range(NDC // 2):
                    mm(
                        ps_k[:],
                        w8qkv[:, 2 * dc2 : 2 * dc2 + 2, D + hp * P : D + (hp + 1) * P],
                        xn1T8[lc][:, 2 * dc2 : 2 * dc2 + 2, :],
                        start=(dc2 == 0),
                        stop=(dc2 == NDC // 2 - 1),
                        perf_mode=DR,
                    )
                act(
                    kT[:, lc * 512 : (lc + 1) * 512],
                    ps_k[:],
                    AF.Identity,
                    bias=bq_col[:, 6 + hp : 7 + hp],
                    scale=1.0 / SW_QKV,
                )
            for dlt in range(2):
                h, off = 2 * hp + dlt, dlt * HD
                ps_o = psO.tile([HD + 1, 512], F32)
                for kc2 in range(NKC // 2):
                    ps_s = psS.tile([P, 1024], F32)
                    for j in range(2):
                        kc = 2 * kc2 + j
                        mm(
                            ps_s[:, j * 512 : (j + 1) * 512],
                            kT[off : off + HD, kc * P : (kc + 1) * P],
                            qT_all[off : off + HD, hp, :],
                            start=True,
                            stop=True,
                        )
                    ptile = pt_pool.tile([P, 1024], BF16)
                    if kc2 % 3 == 2:
                        # Schraudolph exp on VectorE: int16 bits of bf16 e^x
                        v.tensor_scalar(
                            ptile.bitcast(I16)[:], ps_s[:], SCH_A, SCH_B,
                            op0=OP.mult, op1=OP.add,
                        )
                    else:
                        act(ptile[:], ps_s[:], AF.Exp, scale=0.125)
                    for j in range(2):
                        kc = 2 * kc2 + j
                        mm(
                            ps_o[:],
                            v_all[:, kc, h * (HD + 1) : (h + 1) * (HD + 1)],
                            ptile[:, j * 512 : (j + 1) * 512],
                            start=(kc == 0),
                            stop=(kc == NKC - 1),
                        )
                # normalize columns by the ones-row (softmax denominator)
                zrow = tiny.tile([1, 512], F32)
                v.tensor_copy(zrow[:], ps_o[HD : HD + 1, :])
                rz = tiny.tile([1, 512], F32, tag="rz")
                v.reciprocal_approx_fast(rz[:], zrow[:])
                rz_b = rzb_pool.tile([P, 512], F32)
                nc.gpsimd.partition_broadcast(rz_b[:], rz[:])
                v.tensor_tensor(
                    catT8[off : off + HD, hp, :],
                    ps_o[0:HD, :],
                    rz_b[0:HD, :],
                    op=OP.mult,
                )

    s_attn.close()  # free K/V/Q/xn1T space before the FFN weights land

    # -------- phase D: out-projection, residual, LN2 (per-qc fused) ------
    with ExitStack() as phD:
        pool = phD.enter_context(tc.tile_pool(name="phD", bufs=2))
        spool = phD.enter_context(tc.tile_pool(name="spoolE", bufs=3))
        nxpool = phD.enter_context(tc.tile_pool(name="nxE", bufs=2))
        tpool2 = phD.enter_context(tc.tile_pool(name="tpool2", bufs=1))
        psD1 = phD.enter_context(tc.tile_pool(name="psD1", bufs=2, space="PSUM"))
        psD2 = phD.enter_context(tc.tile_pool(name="psD2", bufs=2, space="PSUM"))

        xn2T_bf = tpool2.tile([P, NDC, LQ], BF16)
        for qc in range(NQC):
            ps1 = psD1.tile([P, 512], F32)
            ps2 = psD2.tile([P, 256], F32)
            for cc2 in range(NDC // 2):
                lhs = catT8[:, 2 * cc2 : 2 * cc2 + 2, qc * P : (qc + 1) * P]
                mm(ps1[:], lhs, w8ao[:, 2 * cc2 : 2 * cc2 + 2, 0:512],
                   start=(cc2 == 0), stop=(cc2 == NDC // 2 - 1), perf_mode=DR)
                mm(ps2[:], lhs, w8ao[:, 2 * cc2 : 2 * cc2 + 2, 512:768],
                   start=(cc2 == 0), stop=(cc2 == NDC // 2 - 1), perf_mode=DR)
            # x2 = x + (psum / SW_AO) + g1*b  (bias tile precomputed)
            xq = x2_loc[qc][:]
            v.scalar_tensor_tensor(
                xq[:, 0:512], ps1[:], 1.0 / SW_AO, x_loc[:, qc, 0:512],
                op0=OP.mult, op1=OP.add,
            )
            v.scalar_tensor_tensor(
                xq[:, 512:768], ps2[:], 1.0 / SW_AO, x_loc[:, qc, 512:768],
                op0=OP.mult, op1=OP.add,
            )
            v.tensor_add(xq, xq, xb_bias[:])
            # LN2 for this q-chunk
            stats = spool.tile([P, 2, 6], F32)
            for g in range(2):
                v.bn_stats(stats[:, g, :], xq[:, g * 384 : (g + 1) * 384])
            mv = spool.tile([P, 2], F32)
            v.bn_aggr(mv[:], stats[:])
            sq = spool.tile([P, 1], F32)
            act(sq[:], mv[:, 1:2], AF.Sqrt, bias=eps_t[:, 0:1])
            rstd = spool.tile([P, 1], F32)
            v.reciprocal_approx_fast(rstd[:], sq[:])
            nx = nxpool.tile([P, D], BF16)
            v.tensor_scalar(
                nx[:], xq, mv[:, 0:1], rstd[:], op0=OP.subtract, op1=OP.mult
            )
            dmat(out=xn2T_bf[:, :, qc * P : (qc + 1) * P], in_=nx[:])
        for dc in range(NDC):
            v.tensor_scalar(
                xn2T8[:, dc, :],
                xn2T_bf[:, dc, :],
                sp2_col[:, dc : dc + 1],
                sh2_col[:, dc : dc + 1],
                op0=OP.mult,
                op1=OP.add,
            )

    # ---------------- phase F: FFN + gate + residual -> output -------------
    with ExitStack() as phF:
        wpool = phF.enter_context(tc.tile_pool(name="wffn", bufs=1))
        hpool = phF.enter_context(tc.tile_pool(name="hT", bufs=1))
        pool = phF.enter_context(tc.tile_pool(name="phF", bufs=2))
        psF1 = phF.enter_context(tc.tile_pool(name="psF1", bufs=3, space="PSUM"))
        psF2 = phF.enter_context(tc.tile_pool(name="psF2", bufs=2, space="PSUM"))

        w8f1 = wpool.tile([P, NDC, DM], FP8)
        dma(out=w8f1[:], in_=t["w_ffn18"].rearrange("(c p) m -> p c m", p=P))
        bf1_col = wpool.tile([P, NMC], F32)
        dma(out=bf1_col[:], in_=t["b_ffn1_col"][:])
        wf2_bf = wpool.tile([P, NMC, D], BF16)
        dma(out=wf2_bf[:], in_=t["w_f2_bf"].rearrange("(c p) m -> p c m", p=P))
        w8f2 = wpool.tile([P, NMC, D], FP8)  # w_ffn2 * g2 * SW_F2
        # fold g2 into ffn2 weights -> fp8 (single quantization)
        for c in range(NMC):
            v.tensor_tensor(w8f2[:, c, :], wf2_bf[:, c, :], g2s_b[:], op=OP.mult)

        hT8 = hpool.tile([P, NMC, LQ], FP8)
        for mc in range(NMC):
            ps_h = psF1.tile([P, 512], F32, tag="mm512")
            for dc2 in range(NDC // 2):
                mm(
                    ps_h[:],
                    w8f1[:, 2 * dc2 : 2 * dc2 + 2, mc * P : (mc + 1) * P],
                    xn2T8[:, 2 * dc2 : 2 * dc2 + 2, :],
                    start=(dc2 == 0),
                    stop=(dc2 == NDC // 2 - 1),
                    perf_mode=DR,
                )
            act(
                hT8[:, mc, :], ps_h[:], AF.Gelu,
                bias=bf1_col[:, mc : mc + 1], scale=1.0 / SW_F1,
            )

        out_r = t["out"].rearrange("(n p) d -> n p d", p=P)
        for qc in range(NQC):
            ps1 = psF1.tile([P, 512], F32, tag="mm512")
            ps2 = psF2.tile([P, 256], F32)
            for mc2 in range(NMC // 2):
                lhs = hT8[:, 2 * mc2 : 2 * mc2 + 2, qc * P : (qc + 1) * P]
                mm(ps1[:], lhs, w8f2[:, 2 * mc2 : 2 * mc2 + 2, 0:512],
                   start=(mc2 == 0), stop=(mc2 == NMC // 2 - 1), perf_mode=DR)
                mm(ps2[:], lhs, w8f2[:, 2 * mc2 : 2 * mc2 + 2, 512:768],
                   start=(mc2 == 0), stop=(mc2 == NMC // 2 - 1), perf_mode=DR)
            ot = pool.tile([P, D], F32)
            v.scalar_tensor_tensor(
                ot[:, 0:512], ps1[:], 1.0 / SW_F2, x2_loc[qc][:, 0:512],
                op0=OP.mult, op1=OP.add,
            )
            v.scalar_tensor_tensor(
                ot[:, 512:768], ps2[:], 1.0 / SW_F2, x2_loc[qc][:, 512:768],
                op0=OP.mult, op1=OP.add,
            )
            v.tensor_add(ot[:], ot[:], x2b_bias[:])
            dma(out=out_r[qc], in_=ot[:])


def build_nc():
    nc = bacc.Bacc(None, target_bir_lowering=False, debug=False)
    t = _declare_params(nc)
    with tile.TileContext(nc) as tc:
        with ExitStack() as ctx:
            _build_body(nc, tc, ctx, t)
    nc.compile()
    return nc


_cache = {}


def _prep_in_maps(inputs):
    E4 = ml_dtypes.float8_e4m3fn
    bf = lambda a: np.ascontiguousarray(np.asarray(a, np.float32)).astype(
        ml_dtypes.bfloat16
    )
    f32 = lambda a: np.ascontiguousarray(np.asarray(a, np.float32))
    q8 = lambda a, s: np.ascontiguousarray(
        (np.asarray(a, np.float32) * s).astype(E4)
    )
    x = f32(inputs["x"]).reshape(L, D)
    cond = f32(inputs["cond"]).reshape(D)
    b_adaln1 = f32(inputs["b_adaln1"]).reshape(3 * D)
    b_adaln2 = f32(inputs["b_adaln2"]).reshape(3 * D)
    b_qkv = f32(inputs["b_qkv"]).reshape(3 * D)
    w_ao = f32(inputs["w_attn_out"])
    # fold the V bias through the out-projection: b_attn_eff = b + bv @ Wao
    b_attn_eff = f32(inputs["b_attn_out"]).reshape(D) + b_qkv[2 * D :] @ w_ao
    common = {
        "cond_t": np.ascontiguousarray(cond.reshape(NDC, P).T),
        "w_adaln1": bf(inputs["w_adaln1"]),
        "w_adaln2": bf(inputs["w_adaln2"]),
        "b_adaln1_col": np.ascontiguousarray(b_adaln1[: 12 * P].reshape(12, P).T),
        "b_adaln2_col": np.ascontiguousarray(b_adaln2[: 12 * P].reshape(12, P).T),
        "b_adaln1_gate": np.ascontiguousarray(b_adaln1[2 * D :][None]),
        "b_adaln2_gate": np.ascontiguousarray(b_adaln2[2 * D :][None]),
        "w_qkv8": q8(inputs["w_qkv"], SW_QKV),
        "b_qkv_col": np.ascontiguousarray(b_qkv.reshape(18, P).T),
        "w_ao_bf": bf(w_ao),
        "b_attn_b": np.ascontiguousarray(np.broadcast_to(b_attn_eff, (P, D))),
        "w_ffn18": q8(inputs["w_ffn1"], SW_F1),
        "b_ffn1_col": np.ascontiguousarray(
            f32(inputs["b_ffn1"]).reshape(NMC, P).T
        ),
        "w_f2_bf": bf(inputs["w_ffn2"]),
        "b_ffn2_b": np.ascontiguousarray(
            np.broadcast_to(f32(inputs["b_ffn2"]).reshape(D), (P, D))
        ),
    }
    in_maps = []
    for c in range(NCORES):
        m = dict(common)
        m["x"] = np.ascontiguousarray(np.roll(x, -c * LQ, axis=0))
        in_maps.append(m)
    return in_maps


def kernel(**inputs):
    if "nc" not in _cache:
        _cache["nc"] = build_nc()
    nc = _cache["nc"]
    in_maps = _prep_in_maps(inputs)
    res = run_bass_kernel_spmd(nc, in_maps, list(range(NCORES)))
    out = np.concatenate([res.results[c]["out"] for c in range(NCORES)], axis=0)
    return out.reshape(1, L, D).astype(np.float32)


if __name__ == "__main__":
    rng = np.random.default_rng(0)
    fake = {
        "x": rng.standard_normal((1, L, D), dtype=np.float32),
        "cond": rng.standard_normal((1, D), dtype=np.float32),
        "w_adaln1": rng.standard_normal((D, 3 * D), dtype=np.float32) * 0.02,
        "b_adaln1": np.zeros(3 * D, np.float32),
        "w_qkv": rng.standard_normal((D, 3 * D), dtype=np.float32) * D**-0.5,
        "b_qkv": np.zeros(3 * D, np.float32),
        "w_attn_out": rng.standard_normal((D, D), dtype=np.float32) * D**-0.5,
        "b_attn_out": np.zeros(D, np.float32),
        "w_adaln2": rng.standard_normal((D, 3 * D), dtype=np.float32) * 0.02,
        "b_adaln2": np.zeros(3 * D, np.float32),
        "w_ffn1": rng.standard_normal((D, DM), dtype=np.float32) * D**-0.5,
        "b_ffn1": np.zeros(DM, np.float32),
        "w_ffn2": rng.standard_normal((DM, D), dtype=np.float32) * DM**-0.5,
        "b_ffn2": np.zeros(D, np.float32),
    }
    out = kernel(**fake)
    print("out", out.shape, out.dtype, np.abs(out).max())


# revision 8
# speedup vs baseline: 1.4915x; 1.0307x over previous
"""DiT block kernel for 8 Trainium2 NeuronCores (Bass/Tile).

Sharding: sequence dim L=4096 split 8 ways (512 query rows per core).
Each core computes K/V for the full sequence (replicated; no cross-core
collective) but only its own 512 queries through attention, the
out-projection, and the FFN. Inputs are rotated host-side so every
core's local rows sit at positions [0, 512) -> one SPMD program.

Precision: fp8e4 (e4m3) DoubleRow matmuls for all projections, the FFN
and the attention scores (2x TensorE throughput); bf16 for attn @ V;
fp32 accumulation in PSUM; layernorm stats and residuals fp32.
AdaLN modulation is fused into the bf16->fp8 convert of the transposed
LN output. Softmax exp is split between ScalarE (native Exp) and
VectorE (Schraudolph int16-bitcast approximation).
"""

import sys

sys.path.insert(0, "/opt/trn_rl_repo")

from contextlib import ExitStack

import numpy as np
import ml_dtypes

import concourse.bass as bass
import concourse.bacc as bacc
import concourse.tile as tile
import concourse.mybir as mybir
from concourse.bass_utils import run_bass_kernel_spmd

F32 = mybir.dt.float32
BF16 = mybir.dt.bfloat16
FP8 = mybir.dt.float8e4
I16 = mybir.dt.int16
AF = mybir.ActivationFunctionType
OP = mybir.AluOpType
DR = mybir.MatmulPerfMode.DoubleRow

L, D, H, HD, DM = 4096, 768, 12, 64, 3072
NCORES = 8
LQ = L // NCORES  # 512 local query rows
P = 128
EPS = 1e-5
NLC = L // 512  # 8 l-chunks of 512
NKC = L // P  # 32 k-chunks of 128
NQC = LQ // P  # 4 local q-chunks of 128
NDC = D // P  # 6 chunks of the model dim
NHP = H // 2  # 6 head pairs
NMC = DM // P  # 24 chunks of the FFN hidden dim

# host-side power-of-two scales applied to fp8 weights (exact to undo)
SW_QKV = 64.0
SW_AO = 16.0
SW_F1 = 16.0
SW_F2 = 32.0

# Schraudolph exp -> bf16 bits: i16 = round(x*2^7/ln2 + (127*2^7 - C))
SCH_A = 184.6650390625 * 0.125  # includes the hd^-0.5 = 1/8 score scale
SCH_B = 16248.6


def _declare_params(nc):
    dp = nc.declare_dram_parameter
    t = {}
    t["x"] = dp("x", [L, D], F32, isOutput=False)
    t["cond_t"] = dp("cond_t", [P, NDC], F32, isOutput=False)
    t["w_adaln1"] = dp("w_adaln1", [D, 3 * D], BF16, isOutput=False)
    t["w_adaln2"] = dp("w_adaln2", [D, 3 * D], BF16, isOutput=False)
    t["b_adaln1_col"] = dp("b_adaln1_col", [P, 12], F32, isOutput=False)
    t["b_adaln2_col"] = dp("b_adaln2_col", [P, 12], F32, isOutput=False)
    t["b_adaln1_gate"] = dp("b_adaln1_gate", [1, D], F32, isOutput=False)
    t["b_adaln2_gate"] = dp("b_adaln2_gate", [1, D], F32, isOutput=False)
    t["w_qkv8"] = dp("w_qkv8", [D, 3 * D], FP8, isOutput=False)  # * SW_QKV
    t["b_qkv_col"] = dp("b_qkv_col", [P, 18], F32, isOutput=False)
    t["w_ao_bf"] = dp("w_ao_bf", [D, D], BF16, isOutput=False)
    t["b_attn_b"] = dp("b_attn_b", [P, D], F32, isOutput=False)  # b_attn + bv@Wao
    t["w_ffn18"] = dp("w_ffn18", [D, DM], FP8, isOutput=False)  # * SW_F1
    t["b_ffn1_col"] = dp("b_ffn1_col", [P, NMC], F32, isOutput=False)
    t["w_f2_bf"] = dp("w_f2_bf", [DM, D], BF16, isOutput=False)
    t["b_ffn2_b"] = dp("b_ffn2_b", [P, D], F32, isOutput=False)
    t["out"] = dp("out", [LQ, D], F32, isOutput=True)
    return t


def _build_body(nc, tc, ctx, t):
    mm = nc.tensor.matmul
    dma = nc.sync.dma_start
    dmat = nc.sync.dma_start_transpose
    v = nc.vector
    act = nc.scalar.activation

    const = ctx.enter_context(tc.tile_pool(name="const", bufs=1))
    eps_t = const.tile([P, 1], F32)
    v.memset(eps_t, EPS)

    # ---------------- phase A: cond path (SiLU + AdaLN projections) -------
    adaln = ctx.enter_context(tc.tile_pool(name="adaln", bufs=1))
    sc_bf = adaln.tile([P, NDC], BF16)
    sh1_col = adaln.tile([P, NDC], F32)
    sp1_col = adaln.tile([P, NDC], F32)
    sh2_col = adaln.tile([P, NDC], F32)
    sp2_col = adaln.tile([P, NDC], F32)
    g1s_b = adaln.tile([P, D], F32)  # g1 * SW_AO, broadcast
    g2s_b = adaln.tile([P, D], F32)  # g2 * SW_F2, broadcast
    w8ao = adaln.tile([P, NDC, D], FP8)  # w_attn_out * g1 * SW_AO
    xb_bias = adaln.tile([P, D], F32)  # g1 * b_attn_eff
    x2b_bias = adaln.tile([P, D], F32)  # g2 * b_ffn2

    with ExitStack() as phA:
        pool = phA.enter_context(tc.tile_pool(name="phA", bufs=1))
        psA1 = phA.enter_context(tc.tile_pool(name="psA1", bufs=4, space="PSUM"))
        psA2 = phA.enter_context(tc.tile_pool(name="psA2", bufs=2, space="PSUM"))

        cond_sb = pool.tile([P, NDC], F32)
        dma(out=cond_sb[:], in_=t["cond_t"][:])
        sc_f = pool.tile([P, NDC], F32)
        act(sc_f[:], cond_sb[:], AF.Silu)
        v.tensor_copy(sc_bf[:], sc_f[:])

        wa1 = pool.tile([P, NDC, 3 * D], BF16)
        wa2 = pool.tile([P, NDC, 3 * D], BF16)
        for dc in range(NDC):
            dma(out=wa1[:, dc, :], in_=t["w_adaln1"].rearrange("(c p) m -> p c m", p=P)[:, dc, :])
        for dc in range(NDC):
            dma(out=wa2[:, dc, :], in_=t["w_adaln2"].rearrange("(c p) m -> p c m", p=P)[:, dc, :])
        b1c = pool.tile([P, 12], F32)
        dma(out=b1c[:], in_=t["b_adaln1_col"][:])
        b2c = pool.tile([P, 12], F32)
        dma(out=b2c[:], in_=t["b_adaln2_col"][:])
        b1g = pool.tile([1, D], F32)
        dma(out=b1g[:], in_=t["b_adaln1_gate"][:])
        b2g = pool.tile([1, D], F32)
        dma(out=b2g[:], in_=t["b_adaln2_gate"][:])

        for r, (wa, bc, bg, sh_col, sp_col, gs_b, sscale) in enumerate(
            [
                (wa1, b1c, b1g, sh1_col, sp1_col, g1s_b, SW_AO),
                (wa2, b2c, b2g, sh2_col, sp2_col, g2s_b, SW_F2),
            ]
        ):
            acol = pool.tile([P, 12], F32, name=f"acol{r}")
            for m in range(12):
                ps = psA1.tile([P, 1], F32)
                for dc in range(NDC):
                    mm(
                        ps[:],
                        wa[:, dc, m * P : (m + 1) * P],
                        sc_bf[:, dc : dc + 1],
                        start=(dc == 0),
                        stop=(dc == NDC - 1),
                    )
                v.tensor_add(acol[:, m : m + 1], ps[:], bc[:, m : m + 1])
            v.tensor_copy(sh_col[:], acol[:, 0:6])
            v.tensor_scalar_add(sp_col[:], acol[:, 6:12], 1.0)
            # gate row, scaled by the weight-quant scale, then broadcast
            g_row = pool.tile([1, D], F32, name=f"grow{r}")
            for j, (n0, n1) in enumerate([(0, 512), (512, 768)]):
                ps = psA2.tile([1, n1 - n0], F32, tag="psg")
                for dc in range(NDC):
                    mm(
                        ps[:],
                        sc_bf[:, dc : dc + 1],
                        wa[:, dc, 2 * D + n0 : 2 * D + n1],
                        start=(dc == 0),
                        stop=(dc == NDC - 1),
                    )
                v.tensor_add(g_row[:, n0:n1], ps[:], bg[:, n0:n1])
            gs_row = pool.tile([1, D], F32, name=f"gsrow{r}")
            v.tensor_scalar_mul(gs_row[:], g_row[:], sscale)
            nc.gpsimd.partition_broadcast(gs_b[:], gs_row[:])


    # ---------------- phase B: LN1 -> xn1T (fp8, modulated) + V/Q ---------
    big = ctx.enter_context(tc.tile_pool(name="big", bufs=1))
    x_loc = big.tile([P, NQC, D], F32)  # local rows for the residual
    x2_loc = [big.tile([P, D], F32, name=f"x2_loc{q}") for q in range(NQC)]
    catT8 = big.tile([P, NDC, LQ], FP8)  # attention output, transposed
    xn2T8 = big.tile([P, NDC, LQ], FP8)
    s_attn = ctx.enter_context(ExitStack())
    attn_pool = s_attn.enter_context(tc.tile_pool(name="attn", bufs=1))
    xn1T8 = [
        attn_pool.tile([P, NDC, 512], FP8, name=f"xn1T8_{i}") for i in range(NLC)
    ]
    v_all = attn_pool.tile([P, NKC, H * (HD + 1)], BF16)  # V + ones column
    qT_all = attn_pool.tile([P, NHP, LQ], BF16)
    w8qkv = attn_pool.tile([P, NDC, 3 * D], FP8)
    bq_col = attn_pool.tile([P, 18], F32)

    dma(out=w8qkv[:], in_=t["w_qkv8"].rearrange("(c p) m -> p c m", p=P))
    dma(out=bq_col[:], in_=t["b_qkv_col"][:])
    dma(out=x_loc[:], in_=t["x"][0:LQ, :].rearrange("(n p) d -> p n d", p=P))
    v.memset(
        v_all.rearrange("p k (h e) -> p k h e", e=HD + 1)[:, :, :, HD : HD + 1], 1.0
    )

    with ExitStack() as phB:
        xload = phB.enter_context(tc.tile_pool(name="xload", bufs=4))
        spool = phB.enter_context(tc.tile_pool(name="spool", bufs=6))
        nxpool = phB.enter_context(tc.tile_pool(name="nxpool", bufs=4))
        tpool = phB.enter_context(tc.tile_pool(name="tpool", bufs=2))
        psB1 = phB.enter_context(tc.tile_pool(name="psB1", bufs=2, space="PSUM"))
        psB2 = phB.enter_context(tc.tile_pool(name="psB2", bufs=2, space="PSUM"))

        # LN1 over the full sequence, transposed via DMA-xbar, modulated
        # into fp8 during the convert.
        x_r = t["x"].rearrange("(n p) d -> n p d", p=P)
        xn1T_bf = None
        for i in range(NKC):
            if i % 4 == 0:
                xn1T_bf = tpool.tile([P, NDC, 512], BF16, tag="xnbf")
            xt = xload.tile([P, D], F32)
            dma(out=xt[:], in_=x_r[i])
            stats = spool.tile([P, 2, 6], F32)
            for g in range(2):
                v.bn_stats(stats[:, g, :], xt[:, g * 384 : (g + 1) * 384])
            mv = spool.tile([P, 2], F32)
            v.bn_aggr(mv[:], stats[:])
            sq = spool.tile([P, 1], F32)
            act(sq[:], mv[:, 1:2], AF.Sqrt, bias=eps_t[:, 0:1])
            rstd = spool.tile([P, 1], F32)
            v.reciprocal_approx_fast(rstd[:], sq[:])
            nmr = spool.tile([P, 1], F32)
            v.scalar_tensor_tensor(
                nmr[:], mv[:, 0:1], -1.0, rstd[:], op0=OP.mult, op1=OP.mult
            )
            nx = nxpool.tile([P, D], BF16)
            act(nx[:], xt[:], AF.Identity, bias=nmr[:, 0:1], scale=rstd[:, 0:1])
            dmat(out=xn1T_bf[:, :, (i % 4) * P : (i % 4 + 1) * P], in_=nx[:])
            if i % 4 == 3:
                lc = i // 4
                for dc in range(NDC):
                    v.tensor_scalar(
                        xn1T8[lc][:, dc, :],
                        xn1T_bf[:, dc, :],
                        sp1_col[:, dc : dc + 1],
                        sh1_col[:, dc : dc + 1],
                        op0=OP.mult,
                        op1=OP.add,
                    )

        # V for all heads (fp8 DoubleRow), bf16 result (bias folded away)
        v4 = v_all.rearrange("p k (h e) -> p k h e", e=HD + 1)
        for kc in range(NKC):
            ps_v = psB2.tile([P, D], F32)
            for dc2 in range(NDC // 2):
                lhs = xn1T8[kc // 4][:, 2 * dc2 : 2 * dc2 + 2, (kc % 4) * P : (kc % 4 + 1) * P]
                mm(ps_v[:, 0:512], lhs, w8qkv[:, 2 * dc2 : 2 * dc2 + 2, 2 * D : 2 * D + 512],
                   start=(dc2 == 0), stop=(dc2 == NDC // 2 - 1), perf_mode=DR)
                mm(ps_v[:, 512:768], lhs, w8qkv[:, 2 * dc2 : 2 * dc2 + 2, 2 * D + 512 : 3 * D],
                   start=(dc2 == 0), stop=(dc2 == NDC // 2 - 1), perf_mode=DR)
            if kc % 2 == 0:
                act(
                    v4[:, kc, :, 0:HD],
                    ps_v.rearrange("p (h e) -> p h e", e=HD),
                    AF.Copy,
                    scale=1.0 / SW_QKV,
                )
            else:
                v.tensor_scalar_mul(
                    v4[:, kc, :, 0:HD],
                    ps_v.rearrange("p (h e) -> p h e", e=HD),
                    1.0 / SW_QKV,
                )

        # Q^T bf16 (local rows only)
        for hp in range(NHP):
            ps_q = psB1.tile([P, 512], F32, tag="mm512")
            for dc2 in range(NDC // 2):
                mm(
                    ps_q[:],
                    w8qkv[:, 2 * dc2 : 2 * dc2 + 2, hp * P : (hp + 1) * P],
                    xn1T8[0][:, 2 * dc2 : 2 * dc2 + 2, :],
                    start=(dc2 == 0),
                    stop=(dc2 == NDC // 2 - 1),
                    perf_mode=DR,
                )
            v.tensor_scalar(
                qT_all[:, hp, :], ps_q[:], 1.0 / SW_QKV, bq_col[:, hp : hp + 1],
                op0=OP.mult, op1=OP.add,
            )

    # ------- phase C: merged K-projection + attention pipeline -------------
    with ExitStack() as phC:
        kv_pool = phC.enter_context(tc.tile_pool(name="kvp", bufs=2))
        pt_pool = phC.enter_context(tc.tile_pool(name="ptp", bufs=4))
        tiny = phC.enter_context(tc.tile_pool(name="tiny", bufs=2))
        rzb_pool = phC.enter_context(tc.tile_pool(name="rzb", bufs=2))
        psS = phC.enter_context(tc.tile_pool(name="psS", bufs=2, space="PSUM"))
        psO = phC.enter_context(tc.tile_pool(name="psO", bufs=2, space="PSUM"))
        psK = phC.enter_context(tc.tile_pool(name="psK", bufs=2, space="PSUM"))

        for hp in range(NHP):
            # K^T for this head pair (overlaps with attention on hp-1)
            kT = kv_pool.tile([P, L], BF16, tag="kT")
            for lc in range(NLC):
                ps_k = psK.tile([P, 512], F32)
                for dc2 in range(NDC // 2):
                    mm(
                        ps_k[:],
                        w8qkv[:, 2 * dc2 : 2 * dc2 + 2, D + hp * P : D + (hp + 1) * P],
                        xn1T8[lc][:, 2 * dc2 : 2 * dc2 + 2, :],
                        start=(dc2 == 0),
                        stop=(dc2 == NDC // 2 - 1),
                        perf_mode=DR,
                    )
                act(
                    kT[:, lc * 512 : (lc + 1) * 512],
                    ps_k[:],
                    AF.Identity,
                    bias=bq_col[:, 6 + hp : 7 + hp],
                    scale=1.0 / SW_QKV,
                )
            for dlt in range(2):
                h, off = 2 * hp + dlt, dlt * HD
                ps_o = psO.tile([HD + 1, 512], F32)
                for kc2 in range(NKC // 2):
                    ps_s = psS.tile([P, 1024], F32)
                    for j in range(2):
                        kc = 2 * kc2 + j
                        mm(
                            ps_s[:, j * 512 : (j + 1) * 512],
                            kT[off : off + HD, kc * P : (kc + 1) * P],
                            qT_all[off : off + HD, hp, :],
                            start=True,
                            stop=True,
                        )
                    ptile = pt_pool.tile([P, 1024], BF16)
                    if kc2 % 3 == 2:
                        # Schraudolph exp on VectorE: int16 bits of bf16 e^x
                        v.tensor_scalar(
                            ptile.bitcast(I16)[:], ps_s[:], SCH_A, SCH_B,
                            op0=OP.mult, op1=OP.add,
                        )
                    else:
                        act(ptile[:], ps_s[:], AF.Exp, scale=0.125)
                    for j in range(2):
                        kc = 2 * kc2 + j
                        mm(
                            ps_o[:],
                            v_all[:, kc, h * (HD + 1) : (h + 1) * (HD + 1)],
                            ptile[:, j * 512 : (j + 1) * 512],
                            start=(kc == 0),
                            stop=(kc == NKC - 1),
                        )
                # normalize columns by the ones-row (softmax denominator)
                zrow = tiny.tile([1, 512], F32)
                v.tensor_copy(zrow[:], ps_o[HD : HD + 1, :])
                rz = tiny.tile([1, 512], F32, tag="rz")
                v.reciprocal_approx_fast(rz[:], zrow[:])
                rz_b = rzb_pool.tile([P, 512], F32)
                nc.gpsimd.partition_broadcast(rz_b[:], rz[:])
                v.tensor_tensor(
                    catT8[off : off + HD, hp, :],
                    ps_o[0:HD, :],
                    rz_b[0:HD, :],
                    op=OP.mult,
                )

    s_attn.close()  # free K/V/Q/xn1T space before the FFN weights land

    # -------- phase D: out-projection, residual, LN2 (per-qc fused) ------
    with ExitStack() as phD:
        pool = phD.enter_context(tc.tile_pool(name="phD", bufs=2))
        spool = phD.enter_context(tc.tile_pool(name="spoolE", bufs=3))
        nxpool = phD.enter_context(tc.tile_pool(name="nxE", bufs=2))
        tpool2 = phD.enter_context(tc.tile_pool(name="tpool2", bufs=1))
        psD1 = phD.enter_context(tc.tile_pool(name="psD1", bufs=2, space="PSUM"))
        psD2 = phD.enter_context(tc.tile_pool(name="psD2", bufs=2, space="PSUM"))

        wao_bf = pool.tile([P, NDC, D], BF16, name="wao_bf")
        dma(out=wao_bf[:], in_=t["w_ao_bf"].rearrange("(c p) m -> p c m", p=P))
        ba_sb = pool.tile([P, D], F32, name="ba_sb")
        dma(out=ba_sb[:], in_=t["b_attn_b"][:])
        bf2_sb = pool.tile([P, D], F32, name="bf2_sb")
        dma(out=bf2_sb[:], in_=t["b_ffn2_b"][:])
        # fold gates into the fp8 out-proj weights (one quantization)
        for c in range(NDC):
            v.tensor_tensor(w8ao[:, c, :], wao_bf[:, c, :], g1s_b[:], op=OP.mult)
        v.scalar_tensor_tensor(
            xb_bias[:], ba_sb[:], 1.0 / SW_AO, g1s_b[:], op0=OP.mult, op1=OP.mult
        )
        v.scalar_tensor_tensor(
            x2b_bias[:], bf2_sb[:], 1.0 / SW_F2, g2s_b[:], op0=OP.mult, op1=OP.mult
        )

        xn2T_bf = tpool2.tile([P, NDC, LQ], BF16)
        for qc in range(NQC):
            ps1 = psD1.tile([P, 512], F32)
            ps2 = psD2.tile([P, 256], F32)
            for cc2 in range(NDC // 2):
                lhs = catT8[:, 2 * cc2 : 2 * cc2 + 2, qc * P : (qc + 1) * P]
                mm(ps1[:], lhs, w8ao[:, 2 * cc2 : 2 * cc2 + 2, 0:512],
                   start=(cc2 == 0), stop=(cc2 == NDC // 2 - 1), perf_mode=DR)
                mm(ps2[:], lhs, w8ao[:, 2 * cc2 : 2 * cc2 + 2, 512:768],
                   start=(cc2 == 0), stop=(cc2 == NDC // 2 - 1), perf_mode=DR)
            # x2 = x + (psum / SW_AO) + g1*b  (bias tile precomputed)
            xq = x2_loc[qc][:]
            v.scalar_tensor_tensor(
                xq[:, 0:512], ps1[:], 1.0 / SW_AO, x_loc[:, qc, 0:512],
                op0=OP.mult, op1=OP.add,
            )
            v.scalar_tensor_tensor(
                xq[:, 512:768], ps2[:], 1.0 / SW_AO, x_loc[:, qc, 512:768],
                op0=OP.mult, op1=OP.add,
            )
            v.tensor_add(xq, xq, xb_bias[:])
            # LN2 for this q-chunk
            stats = spool.tile([P, 2, 6], F32)
            for g in range(2):
                v.bn_stats(stats[:, g, :], xq[:, g * 384 : (g + 1) * 384])
            mv = spool.tile([P, 2], F32)
            v.bn_aggr(mv[:], stats[:])
            sq = spool.tile([P, 1], F32)
            act(sq[:], mv[:, 1:2], AF.Sqrt, bias=eps_t[:, 0:1])
            rstd = spool.tile([P, 1], F32)
            v.reciprocal_approx_fast(rstd[:], sq[:])
            nx = nxpool.tile([P, D], BF16)
            v.tensor_scalar(
                nx[:], xq, mv[:, 0:1], rstd[:], op0=OP.subtract, op1=OP.mult
            )
            dmat(out=xn2T_bf[:, :, qc * P : (qc + 1) * P], in_=nx[:])
        for dc in range(NDC):
            v.tensor_scalar(
                xn2T8[:, dc, :],
                xn2T_bf[:, dc, :],
                sp2_col[:, dc : dc + 1],
                sh2_col[:, dc : dc + 1],
                op0=OP.mult,
                op1=OP.add,
            )

    # ---------------- phase F: FFN + gate + residual -> output -------------
    with ExitStack() as phF:
        wpool = phF.enter_context(tc.tile_pool(name="wffn", bufs=1))
        hpool = phF.enter_context(tc.tile_pool(name="hT", bufs=1))
        pool = phF.enter_context(tc.tile_pool(name="phF", bufs=2))
        psF1 = phF.enter_context(tc.tile_pool(name="psF1", bufs=3, space="PSUM"))
        psF2 = phF.enter_context(tc.tile_pool(name="psF2", bufs=2, space="PSUM"))

        w8f1 = wpool.tile([P, NDC, DM], FP8)
        dma(out=w8f1[:], in_=t["w_ffn18"].rearrange("(c p) m -> p c m", p=P))
        bf1_col = wpool.tile([P, NMC], F32)
        dma(out=bf1_col[:], in_=t["b_ffn1_col"][:])
        wf2_bf = wpool.tile([P, NMC, D], BF16)
        dma(out=wf2_bf[:], in_=t["w_f2_bf"].rearrange("(c p) m -> p c m", p=P))
        w8f2 = wpool.tile([P, NMC, D], FP8)  # w_ffn2 * g2 * SW_F2
        # fold g2 into ffn2 weights -> fp8 (single quantization)
        for c in range(NMC):
            v.tensor_tensor(w8f2[:, c, :], wf2_bf[:, c, :], g2s_b[:], op=OP.mult)

        hT8 = hpool.tile([P, NMC, LQ], FP8)
        for mc in range(NMC):
            ps_h = psF1.tile([P, 512], F32, tag="mm512")
            for dc2 in range(NDC // 2):
                mm(
                    ps_h[:],
                    w8f1[:, 2 * dc2 : 2 * dc2 + 2, mc * P : (mc + 1) * P],
                    xn2T8[:, 2 * dc2 : 2 * dc2 + 2, :],
                    start=(dc2 == 0),
                    stop=(dc2 == NDC // 2 - 1),
                    perf_mode=DR,
                )
            act(
                hT8[:, mc, :], ps_h[:], AF.Gelu,
                bias=bf1_col[:, mc : mc + 1], scale=1.0 / SW_F1,
            )

        out_r = t["out"].rearrange("(n p) d -> n p d", p=P)
        for qc in range(NQC):
            ps1 = psF1.tile([P, 512], F32, tag="mm512")
            ps2 = psF2.tile([P, 256], F32)
            for mc2 in range(NMC // 2):
                lhs = hT8[:, 2 * mc2 : 2 * mc2 + 2, qc * P : (qc + 1) * P]
                mm(ps1[:], lhs, w8f2[:, 2 * mc2 : 2 * mc2 + 2, 0:512],
                   start=(mc2 == 0), stop=(mc2 == NMC // 2 - 1), perf_mode=DR)
                mm(ps2[:], lhs, w8f2[:, 2 * mc2 : 2 * mc2 + 2, 512:768],
                   start=(mc2 == 0), stop=(mc2 == NMC // 2 - 1), perf_mode=DR)
            ot = pool.tile([P, D], F32)
            v.scalar_tensor_tensor(
                ot[:, 0:512], ps1[:], 1.0 / SW_F2, x2_loc[qc][:, 0:512],
                op0=OP.mult, op1=OP.add,
            )
            v.scalar_tensor_tensor(
                ot[:, 512:768], ps2[:], 1.0 / SW_F2, x2_loc[qc][:, 512:768],
                op0=OP.mult, op1=OP.add,
            )
            v.tensor_add(ot[:], ot[:], x2b_bias[:])
            dma(out=out_r[qc], in_=ot[:])


def build_nc():
    nc = bacc.Bacc(None, target_bir_lowering=False, debug=False)
    t = _declare_params(nc)
    with tile.TileContext(nc) as tc:
        with ExitStack() as ctx:
            _build_body(nc, tc, ctx, t)
    nc.compile()
    return nc


_cache = {}


def _prep_in_maps(inputs):
    E4 = ml_dtypes.float8_e4m3fn
    bf = lambda a: np.ascontiguousarray(np.asarray(a, np.float32)).astype(
        ml_dtypes.bfloat16
    )
    f32 = lambda a: np.ascontiguousarray(np.asarray(a, np.float32))
    q8 = lambda a, s: np.ascontiguousarray(
        (np.asarray(a, np.float32) * s).astype(E4)
    )
    x = f32(inputs["x"]).reshape(L, D)
    cond = f32(inputs["cond"]).reshape(D)
    b_adaln1 = f32(inputs["b_adaln1"]).reshape(3 * D)
    b_adaln2 = f32(inputs["b_adaln2"]).reshape(3 * D)
    b_qkv = f32(inputs["b_qkv"]).reshape(3 * D)
    w_ao = f32(inputs["w_attn_out"])
    # fold the V bias through the out-projection: b_attn_eff = b + bv @ Wao
    b_attn_eff = f32(inputs["b_attn_out"]).reshape(D) + b_qkv[2 * D :] @ w_ao
    common = {
        "cond_t": np.ascontiguousarray(cond.reshape(NDC, P).T),
        "w_adaln1": bf(inputs["w_adaln1"]),
        "w_adaln2": bf(inputs["w_adaln2"]),
        "b_adaln1_col": np.ascontiguousarray(b_adaln1[: 12 * P].reshape(12, P).T),
        "b_adaln2_col": np.ascontiguousarray(b_adaln2[: 12 * P].reshape(12, P).T),
        "b_adaln1_gate": np.ascontiguousarray(b_adaln1[2 * D :][None]),
        "b_adaln2_gate": np.ascontiguousarray(b_adaln2[2 * D :][None]),
        "w_qkv8": q8(inputs["w_qkv"], SW_QKV),
        "b_qkv_col": np.ascontiguousarray(b_qkv.reshape(18, P).T),
        "w_ao_bf": bf(w_ao),
        "b_attn_b": np.ascontiguousarray(np.broadcast_to(b_attn_eff, (P, D))),
        "w_ffn18": q8(inputs["w_ffn1"], SW_F1),
        "b_ffn1_col": np.ascontiguousarray(
            f32(inputs["b_ffn1"]).reshape(NMC, P).T
        ),
        "w_f2_bf": bf(inputs["w_ffn2"]),
        "b_ffn2_b": np.ascontiguousarray(
            np.broadcast_to(f32(inputs["b_ffn2"]).reshape(D), (P, D))
        ),
    }
    in_maps = []
    for c in range(NCORES):
        m = dict(common)
        m["x"] = np.ascontiguousarray(np.roll(x, -c * LQ, axis=0))
        in_maps.append(m)
    return in_maps


def kernel(**inputs):
    if "nc" not in _cache:
        _cache["nc"] = build_nc()
    nc = _cache["nc"]
    in_maps = _prep_in_maps(inputs)
    res = run_bass_kernel_spmd(nc, in_maps, list(range(NCORES)))
    out = np.concatenate([res.results[c]["out"] for c in range(NCORES)], axis=0)
    return out.reshape(1, L, D).astype(np.float32)


if __name__ == "__main__":
    rng = np.random.default_rng(0)
    fake = {
        "x": rng.standard_normal((1, L, D), dtype=np.float32),
        "cond": rng.standard_normal((1, D), dtype=np.float32),
        "w_adaln1": rng.standard_normal((D, 3 * D), dtype=np.float32) * 0.02,
        "b_adaln1": np.zeros(3 * D, np.float32),
        "w_qkv": rng.standard_normal((D, 3 * D), dtype=np.float32) * D**-0.5,
        "b_qkv": np.zeros(3 * D, np.float32),
        "w_attn_out": rng.standard_normal((D, D), dtype=np.float32) * D**-0.5,
        "b_attn_out": np.zeros(D, np.float32),
        "w_adaln2": rng.standard_normal((D, 3 * D), dtype=np.float32) * 0.02,
        "b_adaln2": np.zeros(3 * D, np.float32),
        "w_ffn1": rng.standard_normal((D, DM), dtype=np.float32) * D**-0.5,
        "b_ffn1": np.zeros(DM, np.float32),
        "w_ffn2": rng.standard_normal((DM, D), dtype=np.float32) * DM**-0.5,
        "b_ffn2": np.zeros(D, np.float32),
    }
    out = kernel(**fake)
    print("out", out.shape, out.dtype, np.abs(out).max())


# revision 9
# speedup vs baseline: 1.5584x; 1.0448x over previous
"""DiT block kernel for 8 Trainium2 NeuronCores (Bass/Tile).

Sharding: sequence dim L=4096 split 8 ways (512 query rows per core).
Each core computes K/V for the full sequence (replicated; no cross-core
collective) but only its own 512 queries through attention, the
out-projection, and the FFN. Inputs are rotated host-side so every
core's local rows sit at positions [0, 512) -> one SPMD program.

Precision: fp8e4 (e4m3) DoubleRow matmuls for all projections, the FFN
and the attention scores (2x TensorE throughput); bf16 for attn @ V;
fp32 accumulation in PSUM; layernorm stats and residuals fp32.
AdaLN modulation is fused into the bf16->fp8 convert of the transposed
LN output. Softmax exp is split between ScalarE (native Exp) and
VectorE (Schraudolph int16-bitcast approximation).
"""

import sys

sys.path.insert(0, "/opt/trn_rl_repo")

from contextlib import ExitStack

import numpy as np
import ml_dtypes

import concourse.bass as bass
import concourse.bacc as bacc
import concourse.tile as tile
import concourse.mybir as mybir
from concourse.bass_utils import run_bass_kernel_spmd

F32 = mybir.dt.float32
BF16 = mybir.dt.bfloat16
FP8 = mybir.dt.float8e4
I16 = mybir.dt.int16
AF = mybir.ActivationFunctionType
OP = mybir.AluOpType
DR = mybir.MatmulPerfMode.DoubleRow

L, D, H, HD, DM = 4096, 768, 12, 64, 3072
NCORES = 8
LQ = L // NCORES  # 512 local query rows
P = 128
EPS = 1e-5
NLC = L // 512  # 8 l-chunks of 512
NKC = L // P  # 32 k-chunks of 128
NQC = LQ // P  # 4 local q-chunks of 128
NDC = D // P  # 6 chunks of the model dim
NHP = H // 2  # 6 head pairs
NMC = DM // P  # 24 chunks of the FFN hidden dim

# host-side power-of-two scales applied to fp8 weights (exact to undo)
SW_QKV = 64.0
SW_AO = 16.0
SW_F1 = 16.0
SW_F2 = 32.0

# Schraudolph exp -> bf16 bits: i16 = round(x*2^7/ln2 + (127*2^7 - C))
SCH_A = 184.6650390625 * 0.125  # includes the hd^-0.5 = 1/8 score scale
SCH_B = 16248.6


def _declare_params(nc):
    dp = nc.declare_dram_parameter
    t = {}
    t["x"] = dp("x", [LQ, D], F32, isOutput=False)
    t["x_bf"] = dp("x_bf", [L, D], BF16, isOutput=False)
    t["cond_t"] = dp("cond_t", [P, NDC], F32, isOutput=False)
    t["w_adaln1"] = dp("w_adaln1", [D, 3 * D], BF16, isOutput=False)
    t["w_adaln2"] = dp("w_adaln2", [D, 3 * D], BF16, isOutput=False)
    t["b_adaln1_col"] = dp("b_adaln1_col", [P, 12], F32, isOutput=False)
    t["b_adaln2_col"] = dp("b_adaln2_col", [P, 12], F32, isOutput=False)
    t["b_adaln1_gate"] = dp("b_adaln1_gate", [1, D], F32, isOutput=False)
    t["b_adaln2_gate"] = dp("b_adaln2_gate", [1, D], F32, isOutput=False)
    t["w_qkv8"] = dp("w_qkv8", [D, 3 * D], FP8, isOutput=False)  # * SW_QKV
    t["b_qkv_col"] = dp("b_qkv_col", [P, 18], F32, isOutput=False)
    t["w_ao8"] = dp("w_ao8", [D, D], FP8, isOutput=False)  # * SW_AO
    t["b_attn_b"] = dp("b_attn_b", [P, D], F32, isOutput=False)  # b_attn + bv@Wao
    t["w_ffn18"] = dp("w_ffn18", [D, DM], FP8, isOutput=False)  # * SW_F1
    t["b_ffn1_col"] = dp("b_ffn1_col", [P, NMC], F32, isOutput=False)
    t["w_f28"] = dp("w_f28", [DM, D], FP8, isOutput=False)  # * SW_F2
    t["b_ffn2_b"] = dp("b_ffn2_b", [P, D], F32, isOutput=False)
    t["out"] = dp("out", [LQ, D], F32, isOutput=True)
    return t


def _build_body(nc, tc, ctx, t):
    mm = nc.tensor.matmul
    dma = nc.sync.dma_start
    dmat = nc.sync.dma_start_transpose
    v = nc.vector
    act = nc.scalar.activation

    const = ctx.enter_context(tc.tile_pool(name="const", bufs=1))
    eps_t = const.tile([P, 1], F32)
    v.memset(eps_t, EPS)

    # ---------------- phase A: cond path (SiLU + AdaLN projections) -------
    adaln = ctx.enter_context(tc.tile_pool(name="adaln", bufs=1))
    sc_bf = adaln.tile([P, NDC], BF16)
    sh1_col = adaln.tile([P, NDC], F32)
    sp1_col = adaln.tile([P, NDC], F32)
    sh2_col = adaln.tile([P, NDC], F32)
    sp2_col = adaln.tile([P, NDC], F32)
    g1s_b = adaln.tile([P, D], F32)  # g1, broadcast
    g2s_b = adaln.tile([P, D], F32)  # g2, broadcast
    xb_bias = adaln.tile([P, D], F32)  # g1 * b_attn_eff
    x2b_bias = adaln.tile([P, D], F32)  # g2 * b_ffn2

    with ExitStack() as phA:
        pool = phA.enter_context(tc.tile_pool(name="phA", bufs=1))
        psA1 = phA.enter_context(tc.tile_pool(name="psA1", bufs=4, space="PSUM"))
        psA2 = phA.enter_context(tc.tile_pool(name="psA2", bufs=2, space="PSUM"))

        cond_sb = pool.tile([P, NDC], F32)
        dma(out=cond_sb[:], in_=t["cond_t"][:])
        sc_f = pool.tile([P, NDC], F32)
        act(sc_f[:], cond_sb[:], AF.Silu)
        v.tensor_copy(sc_bf[:], sc_f[:])

        wa1 = pool.tile([P, NDC, 3 * D], BF16)
        wa2 = pool.tile([P, NDC, 3 * D], BF16)
        for dc in range(NDC):
            dma(out=wa1[:, dc, :], in_=t["w_adaln1"].rearrange("(c p) m -> p c m", p=P)[:, dc, :])
        for dc in range(NDC):
            dma(out=wa2[:, dc, :], in_=t["w_adaln2"].rearrange("(c p) m -> p c m", p=P)[:, dc, :])
        b1c = pool.tile([P, 12], F32)
        dma(out=b1c[:], in_=t["b_adaln1_col"][:])
        b2c = pool.tile([P, 12], F32)
        dma(out=b2c[:], in_=t["b_adaln2_col"][:])
        b1g = pool.tile([1, D], F32)
        dma(out=b1g[:], in_=t["b_adaln1_gate"][:])
        b2g = pool.tile([1, D], F32)
        dma(out=b2g[:], in_=t["b_adaln2_gate"][:])

        for r, (wa, bc, bg, sh_col, sp_col, gs_b, sscale) in enumerate(
            [
                (wa1, b1c, b1g, sh1_col, sp1_col, g1s_b, 1.0),
                (wa2, b2c, b2g, sh2_col, sp2_col, g2s_b, 1.0),
            ]
        ):
            acol = pool.tile([P, 12], F32, name=f"acol{r}")
            for m in range(12):
                ps = psA1.tile([P, 1], F32)
                for dc in range(NDC):
                    mm(
                        ps[:],
                        wa[:, dc, m * P : (m + 1) * P],
                        sc_bf[:, dc : dc + 1],
                        start=(dc == 0),
                        stop=(dc == NDC - 1),
                    )
                v.tensor_add(acol[:, m : m + 1], ps[:], bc[:, m : m + 1])
            v.tensor_copy(sh_col[:], acol[:, 0:6])
            v.tensor_scalar_add(sp_col[:], acol[:, 6:12], 1.0)
            # gate row, scaled by the weight-quant scale, then broadcast
            g_row = pool.tile([1, D], F32, name=f"grow{r}")
            for j, (n0, n1) in enumerate([(0, 512), (512, 768)]):
                ps = psA2.tile([1, n1 - n0], F32, tag="psg")
                for dc in range(NDC):
                    mm(
                        ps[:],
                        sc_bf[:, dc : dc + 1],
                        wa[:, dc, 2 * D + n0 : 2 * D + n1],
                        start=(dc == 0),
                        stop=(dc == NDC - 1),
                    )
                v.tensor_add(g_row[:, n0:n1], ps[:], bg[:, n0:n1])
            nc.gpsimd.partition_broadcast(gs_b[:], g_row[:])


    # ---------------- phase B: LN1 -> xn1T (fp8, modulated) + V/Q ---------
    big = ctx.enter_context(tc.tile_pool(name="big", bufs=1))
    x_loc = big.tile([P, NQC, D], F32)  # local rows for the residual
    x2_loc = [big.tile([P, D], F32, name=f"x2_loc{q}") for q in range(NQC)]
    catT8 = big.tile([P, NDC, LQ], FP8)  # attention output, transposed
    xn2T8 = big.tile([P, NDC, LQ], FP8)
    s_attn = ctx.enter_context(ExitStack())
    attn_pool = s_attn.enter_context(tc.tile_pool(name="attn", bufs=1))
    xn1T8 = [
        attn_pool.tile([P, NDC, 512], FP8, name=f"xn1T8_{i}") for i in range(NLC)
    ]
    v_all = attn_pool.tile([P, NKC, H * (HD + 1)], BF16)  # V + ones column
    qT_all = attn_pool.tile([P, NHP, LQ], BF16)
    w8qkv = attn_pool.tile([P, NDC, 3 * D], FP8)
    bq_col = attn_pool.tile([P, 18], F32)

    dma(out=w8qkv[:], in_=t["w_qkv8"].rearrange("(c p) m -> p c m", p=P))
    dma(out=bq_col[:], in_=t["b_qkv_col"][:])
    dma(out=x_loc[:], in_=t["x"].rearrange("(n p) d -> p n d", p=P))
    v.memset(
        v_all.rearrange("p k (h e) -> p k h e", e=HD + 1)[:, :, :, HD : HD + 1], 1.0
    )

    with ExitStack() as phB:
        xload = phB.enter_context(tc.tile_pool(name="xload", bufs=4))
        spool = phB.enter_context(tc.tile_pool(name="spool", bufs=6))
        nxpool = phB.enter_context(tc.tile_pool(name="nxpool", bufs=4))
        tpool = phB.enter_context(tc.tile_pool(name="tpool", bufs=2))
        psB1 = phB.enter_context(tc.tile_pool(name="psB1", bufs=2, space="PSUM"))
        psB2 = phB.enter_context(tc.tile_pool(name="psB2", bufs=2, space="PSUM"))

        # LN1 over the full sequence, transposed via DMA-xbar, modulated
        # into fp8 during the convert.
        x_r = t["x_bf"].rearrange("(n p) d -> n p d", p=P)
        xn1T_bf = None
        for i in range(NKC):
            if i % 4 == 0:
                xn1T_bf = tpool.tile([P, NDC, 512], BF16, tag="xnbf")
            xt = xload.tile([P, D], BF16)
            nc.gpsimd.dma_start(out=xt[:], in_=x_r[i])
            stats = spool.tile([P, 2, 6], F32)
            for g in range(2):
                v.bn_stats(stats[:, g, :], xt[:, g * 384 : (g + 1) * 384])
            mv = spool.tile([P, 2], F32)
            v.bn_aggr(mv[:], stats[:])
            sq = spool.tile([P, 1], F32)
            act(sq[:], mv[:, 1:2], AF.Sqrt, bias=eps_t[:, 0:1])
            rstd = spool.tile([P, 1], F32)
            v.reciprocal_approx_fast(rstd[:], sq[:])
            nmr = spool.tile([P, 1], F32)
            v.scalar_tensor_tensor(
                nmr[:], mv[:, 0:1], -1.0, rstd[:], op0=OP.mult, op1=OP.mult
            )
            nx = nxpool.tile([P, D], BF16)
            act(nx[:], xt[:], AF.Identity, bias=nmr[:, 0:1], scale=rstd[:, 0:1])
            dmat(out=xn1T_bf[:, :, (i % 4) * P : (i % 4 + 1) * P], in_=nx[:])
            if i % 4 == 3:
                lc = i // 4
                for dc in range(NDC):
                    v.tensor_scalar(
                        xn1T8[lc][:, dc, :],
                        xn1T_bf[:, dc, :],
                        sp1_col[:, dc : dc + 1],
                        sh1_col[:, dc : dc + 1],
                        op0=OP.mult,
                        op1=OP.add,
                    )

        # V for all heads (fp8 DoubleRow), bf16 result (bias folded away)
        v4 = v_all.rearrange("p k (h e) -> p k h e", e=HD + 1)
        for kc in range(NKC):
            ps_v = psB2.tile([P, D], F32)
            for dc2 in range(NDC // 2):
                lhs = xn1T8[kc // 4][:, 2 * dc2 : 2 * dc2 + 2, (kc % 4) * P : (kc % 4 + 1) * P]
                mm(ps_v[:, 0:512], lhs, w8qkv[:, 2 * dc2 : 2 * dc2 + 2, 2 * D : 2 * D + 512],
                   start=(dc2 == 0), stop=(dc2 == NDC // 2 - 1), perf_mode=DR)
                mm(ps_v[:, 512:768], lhs, w8qkv[:, 2 * dc2 : 2 * dc2 + 2, 2 * D + 512 : 3 * D],
                   start=(dc2 == 0), stop=(dc2 == NDC // 2 - 1), perf_mode=DR)
            if kc % 2 == 0:
                act(
                    v4[:, kc, :, 0:HD],
                    ps_v.rearrange("p (h e) -> p h e", e=HD),
                    AF.Copy,
                    scale=1.0 / SW_QKV,
                )
            else:
                v.tensor_scalar_mul(
                    v4[:, kc, :, 0:HD],
                    ps_v.rearrange("p (h e) -> p h e", e=HD),
                    1.0 / SW_QKV,
                )

        # Q^T bf16 (local rows only)
        for hp in range(NHP):
            ps_q = psB1.tile([P, 512], F32, tag="mm512")
            for dc2 in range(NDC // 2):
                mm(
                    ps_q[:],
                    w8qkv[:, 2 * dc2 : 2 * dc2 + 2, hp * P : (hp + 1) * P],
                    xn1T8[0][:, 2 * dc2 : 2 * dc2 + 2, :],
                    start=(dc2 == 0),
                    stop=(dc2 == NDC // 2 - 1),
                    perf_mode=DR,
                )
            v.tensor_scalar(
                qT_all[:, hp, :], ps_q[:], 1.0 / SW_QKV, bq_col[:, hp : hp + 1],
                op0=OP.mult, op1=OP.add,
            )

    # ------- phase C: merged K-projection + attention pipeline -------------
    with ExitStack() as phC:
        kv_pool = phC.enter_context(tc.tile_pool(name="kvp", bufs=2))
        pt_pool = phC.enter_context(tc.tile_pool(name="ptp", bufs=4))
        tiny = phC.enter_context(tc.tile_pool(name="tiny", bufs=2))
        rzb_pool = phC.enter_context(tc.tile_pool(name="rzb", bufs=2))
        psS = phC.enter_context(tc.tile_pool(name="psS", bufs=2, space="PSUM"))
        psO = phC.enter_context(tc.tile_pool(name="psO", bufs=2, space="PSUM"))
        psK = phC.enter_context(tc.tile_pool(name="psK", bufs=2, space="PSUM"))

        for hp in range(NHP):
            # K^T for this head pair (overlaps with attention on hp-1)
            kT = kv_pool.tile([P, L], BF16, tag="kT")
            for lc in range(NLC):
                ps_k = psK.tile([P, 512], F32)
                for dc2 in range(NDC // 2):
                    mm(
                        ps_k[:],
                        w8qkv[:, 2 * dc2 : 2 * dc2 + 2, D + hp * P : D + (hp + 1) * P],
                        xn1T8[lc][:, 2 * dc2 : 2 * dc2 + 2, :],
                        start=(dc2 == 0),
                        stop=(dc2 == NDC // 2 - 1),
                        perf_mode=DR,
                    )
                act(
                    kT[:, lc * 512 : (lc + 1) * 512],
                    ps_k[:],
                    AF.Identity,
                    bias=bq_col[:, 6 + hp : 7 + hp],
                    scale=1.0 / SW_QKV,
                )
            for dlt in range(2):
                h, off = 2 * hp + dlt, dlt * HD
                ps_o = psO.tile([HD + 1, 512], F32)
                for kc2 in range(NKC // 2):
                    ps_s = psS.tile([P, 1024], F32)
                    for j in range(2):
                        kc = 2 * kc2 + j
                        mm(
                            ps_s[:, j * 512 : (j + 1) * 512],
                            kT[off : off + HD, kc * P : (kc + 1) * P],
                            qT_all[off : off + HD, hp, :],
                            start=True,
                            stop=True,
                        )
                    ptile = pt_pool.tile([P, 1024], BF16)
                    if kc2 % 3 == 2:
                        # Schraudolph exp on VectorE: int16 bits of bf16 e^x
                        v.tensor_scalar(
                            ptile.bitcast(I16)[:], ps_s[:], SCH_A, SCH_B,
                            op0=OP.mult, op1=OP.add,
                        )
                    else:
                        act(ptile[:], ps_s[:], AF.Exp, scale=0.125)
                    for j in range(2):
                        kc = 2 * kc2 + j
                        mm(
                            ps_o[:],
                            v_all[:, kc, h * (HD + 1) : (h + 1) * (HD + 1)],
                            ptile[:, j * 512 : (j + 1) * 512],
                            start=(kc == 0),
                            stop=(kc == NKC - 1),
                        )
                # normalize columns by the ones-row (softmax denominator)
                zrow = tiny.tile([1, 512], F32)
                v.tensor_copy(zrow[:], ps_o[HD : HD + 1, :])
                rz = tiny.tile([1, 512], F32, tag="rz")
                v.reciprocal_approx_fast(rz[:], zrow[:])
                rz_b = rzb_pool.tile([P, 512], F32)
                nc.gpsimd.partition_broadcast(rz_b[:], rz[:])
                v.tensor_tensor(
                    catT8[off : off + HD, hp, :],
                    ps_o[0:HD, :],
                    rz_b[0:HD, :],
                    op=OP.mult,
                )

    s_attn.close()  # free K/V/Q/xn1T space before the FFN weights land

    # -------- phase D: out-projection, residual, LN2 (per-qc fused) ------
    with ExitStack() as phD:
        pool = phD.enter_context(tc.tile_pool(name="phD", bufs=2))
        spool = phD.enter_context(tc.tile_pool(name="spoolE", bufs=3))
        nxpool = phD.enter_context(tc.tile_pool(name="nxE", bufs=2))
        tpool2 = phD.enter_context(tc.tile_pool(name="tpool2", bufs=1))
        psD1 = phD.enter_context(tc.tile_pool(name="psD1", bufs=2, space="PSUM"))
        psD2 = phD.enter_context(tc.tile_pool(name="psD2", bufs=2, space="PSUM"))

        w8ao = pool.tile([P, NDC, D], FP8, name="w8ao")
        dma(out=w8ao[:], in_=t["w_ao8"].rearrange("(c p) m -> p c m", p=P))
        ba_sb = pool.tile([P, D], F32, name="ba_sb")
        dma(out=ba_sb[:], in_=t["b_attn_b"][:])
        bf2_sb = pool.tile([P, D], F32, name="bf2_sb")
        dma(out=bf2_sb[:], in_=t["b_ffn2_b"][:])
        v.tensor_tensor(xb_bias[:], ba_sb[:], g1s_b[:], op=OP.mult)
        v.tensor_tensor(x2b_bias[:], bf2_sb[:], g2s_b[:], op=OP.mult)
        # xbl = x + g1*b_attn_eff, the per-qc residual base
        xbl = [pool.tile([P, D], F32, name=f"xbl{q}") for q in range(NQC)]
        for q in range(NQC):
            v.tensor_add(xbl[q][:], x_loc[:, q, :], xb_bias[:])

        xn2T_bf = tpool2.tile([P, NDC, LQ], BF16)
        for qc in range(NQC):
            ps1 = psD1.tile([P, 512], F32)
            ps2 = psD2.tile([P, 256], F32)
            for cc2 in range(NDC // 2):
                lhs = catT8[:, 2 * cc2 : 2 * cc2 + 2, qc * P : (qc + 1) * P]
                mm(ps1[:], lhs, w8ao[:, 2 * cc2 : 2 * cc2 + 2, 0:512],
                   start=(cc2 == 0), stop=(cc2 == NDC // 2 - 1), perf_mode=DR)
                mm(ps2[:], lhs, w8ao[:, 2 * cc2 : 2 * cc2 + 2, 512:768],
                   start=(cc2 == 0), stop=(cc2 == NDC // 2 - 1), perf_mode=DR)
            # x2 = x + g1*(psum / SW_AO) + g1*b
            xq = x2_loc[qc][:]
            gt = pool.tile([P, D], F32, tag="gt")
            v.scalar_tensor_tensor(
                gt[:, 0:512], ps1[:], 1.0 / SW_AO, g1s_b[:, 0:512],
                op0=OP.mult, op1=OP.mult,
            )
            v.scalar_tensor_tensor(
                gt[:, 512:768], ps2[:], 1.0 / SW_AO, g1s_b[:, 512:768],
                op0=OP.mult, op1=OP.mult,
            )
            v.tensor_add(xq, gt[:], xbl[qc][:])
            # LN2 for this q-chunk
            stats = spool.tile([P, 2, 6], F32)
            for g in range(2):
                v.bn_stats(stats[:, g, :], xq[:, g * 384 : (g + 1) * 384])
            mv = spool.tile([P, 2], F32)
            v.bn_aggr(mv[:], stats[:])
            sq = spool.tile([P, 1], F32)
            act(sq[:], mv[:, 1:2], AF.Sqrt, bias=eps_t[:, 0:1])
            rstd = spool.tile([P, 1], F32)
            v.reciprocal_approx_fast(rstd[:], sq[:])
            nx = nxpool.tile([P, D], BF16)
            v.tensor_scalar(
                nx[:], xq, mv[:, 0:1], rstd[:], op0=OP.subtract, op1=OP.mult
            )
            dmat(out=xn2T_bf[:, :, qc * P : (qc + 1) * P], in_=nx[:])
        for dc in range(NDC):
            v.tensor_scalar(
                xn2T8[:, dc, :],
                xn2T_bf[:, dc, :],
                sp2_col[:, dc : dc + 1],
                sh2_col[:, dc : dc + 1],
                op0=OP.mult,
                op1=OP.add,
            )

    # ---------------- phase F: FFN + gate + residual -> output -------------
    with ExitStack() as phF:
        wpool = phF.enter_context(tc.tile_pool(name="wffn", bufs=1))
        hpool = phF.enter_context(tc.tile_pool(name="hT", bufs=1))
        pool = phF.enter_context(tc.tile_pool(name="phF", bufs=2))
        psF1 = phF.enter_context(tc.tile_pool(name="psF1", bufs=3, space="PSUM"))
        psF2 = phF.enter_context(tc.tile_pool(name="psF2", bufs=2, space="PSUM"))

        w8f1 = wpool.tile([P, NDC, DM], FP8)
        dma(out=w8f1[:], in_=t["w_ffn18"].rearrange("(c p) m -> p c m", p=P))
        bf1_col = wpool.tile([P, NMC], F32)
        dma(out=bf1_col[:], in_=t["b_ffn1_col"][:])
        w8f2 = wpool.tile([P, NMC, D], FP8)
        dma(out=w8f2[:], in_=t["w_f28"].rearrange("(c p) m -> p c m", p=P))
        # x2 with the gated ffn2 bias folded in (per-qc residual base)
        for q in range(NQC):
            v.tensor_add(x2_loc[q][:], x2_loc[q][:], x2b_bias[:])

        hT8 = hpool.tile([P, NMC, LQ], FP8)
        for mc in range(NMC):
            ps_h = psF1.tile([P, 512], F32, tag="mm512")
            for dc2 in range(NDC // 2):
                mm(
                    ps_h[:],
                    w8f1[:, 2 * dc2 : 2 * dc2 + 2, mc * P : (mc + 1) * P],
                    xn2T8[:, 2 * dc2 : 2 * dc2 + 2, :],
                    start=(dc2 == 0),
                    stop=(dc2 == NDC // 2 - 1),
                    perf_mode=DR,
                )
            act(
                hT8[:, mc, :], ps_h[:], AF.Gelu,
                bias=bf1_col[:, mc : mc + 1], scale=1.0 / SW_F1,
            )

        out_r = t["out"].rearrange("(n p) d -> n p d", p=P)
        for qc in range(NQC):
            ps1 = psF1.tile([P, 512], F32, tag="mm512")
            ps2 = psF2.tile([P, 256], F32)
            for mc2 in range(NMC // 2):
                lhs = hT8[:, 2 * mc2 : 2 * mc2 + 2, qc * P : (qc + 1) * P]
                mm(ps1[:], lhs, w8f2[:, 2 * mc2 : 2 * mc2 + 2, 0:512],
                   start=(mc2 == 0), stop=(mc2 == NMC // 2 - 1), perf_mode=DR)
                mm(ps2[:], lhs, w8f2[:, 2 * mc2 : 2 * mc2 + 2, 512:768],
                   start=(mc2 == 0), stop=(mc2 == NMC // 2 - 1), perf_mode=DR)
            gt = pool.tile([P, D], F32, tag="gt")
            v.scalar_tensor_tensor(
                gt[:, 0:512], ps1[:], 1.0 / SW_F2, g2s_b[:, 0:512],
                op0=OP.mult, op1=OP.mult,
            )
            v.scalar_tensor_tensor(
                gt[:, 512:768], ps2[:], 1.0 / SW_F2, g2s_b[:, 512:768],
                op0=OP.mult, op1=OP.mult,
            )
            ot = pool.tile([P, D], F32)
            v.tensor_add(ot[:], gt[:], x2_loc[qc][:])
            dma(out=out_r[qc], in_=ot[:])


def build_nc():
    nc = bacc.Bacc(None, target_bir_lowering=False, debug=False)
    t = _declare_params(nc)
    with tile.TileContext(nc) as tc:
        with ExitStack() as ctx:
            _build_body(nc, tc, ctx, t)
    nc.compile()
    return nc


_cache = {}


def _prep_in_maps(inputs):
    E4 = ml_dtypes.float8_e4m3fn
    bf = lambda a: np.ascontiguousarray(np.asarray(a, np.float32)).astype(
        ml_dtypes.bfloat16
    )
    f32 = lambda a: np.ascontiguousarray(np.asarray(a, np.float32))
    q8 = lambda a, s: np.ascontiguousarray(
        (np.asarray(a, np.float32) * s).astype(E4)
    )
    x = f32(inputs["x"]).reshape(L, D)
    cond = f32(inputs["cond"]).reshape(D)
    b_adaln1 = f32(inputs["b_adaln1"]).reshape(3 * D)
    b_adaln2 = f32(inputs["b_adaln2"]).reshape(3 * D)
    b_qkv = f32(inputs["b_qkv"]).reshape(3 * D)
    w_ao = f32(inputs["w_attn_out"])
    # fold the V bias through the out-projection: b_attn_eff = b + bv @ Wao
    b_attn_eff = f32(inputs["b_attn_out"]).reshape(D) + b_qkv[2 * D :] @ w_ao
    common = {
        "cond_t": np.ascontiguousarray(cond.reshape(NDC, P).T),
        "w_adaln1": bf(inputs["w_adaln1"]),
        "w_adaln2": bf(inputs["w_adaln2"]),
        "b_adaln1_col": np.ascontiguousarray(b_adaln1[: 12 * P].reshape(12, P).T),
        "b_adaln2_col": np.ascontiguousarray(b_adaln2[: 12 * P].reshape(12, P).T),
        "b_adaln1_gate": np.ascontiguousarray(b_adaln1[2 * D :][None]),
        "b_adaln2_gate": np.ascontiguousarray(b_adaln2[2 * D :][None]),
        "w_qkv8": q8(inputs["w_qkv"], SW_QKV),
        "b_qkv_col": np.ascontiguousarray(b_qkv.reshape(18, P).T),
        "w_ao8": q8(w_ao, SW_AO),
        "b_attn_b": np.ascontiguousarray(np.broadcast_to(b_attn_eff, (P, D))),
        "w_ffn18": q8(inputs["w_ffn1"], SW_F1),
        "b_ffn1_col": np.ascontiguousarray(
            f32(inputs["b_ffn1"]).reshape(NMC, P).T
        ),
        "w_f28": q8(inputs["w_ffn2"], SW_F2),
        "b_ffn2_b": np.ascontiguousarray(
            np.broadcast_to(f32(inputs["b_ffn2"]).reshape(D), (P, D))
        ),
    }
    in_maps = []
    for c in range(NCORES):
        m = dict(common)
        xr = np.roll(x, -c * LQ, axis=0)
        m["x"] = np.ascontiguousarray(xr[:LQ])
        m["x_bf"] = np.ascontiguousarray(xr.astype(ml_dtypes.bfloat16))
        in_maps.append(m)
    return in_maps


def kernel(**inputs):
    if "nc" not in _cache:
        _cache["nc"] = build_nc()
    nc = _cache["nc"]
    in_maps = _prep_in_maps(inputs)
    res = run_bass_kernel_spmd(nc, in_maps, list(range(NCORES)))
    out = np.concatenate([res.results[c]["out"] for c in range(NCORES)], axis=0)
    return out.reshape(1, L, D).astype(np.float32)


if __name__ == "__main__":
    rng = np.random.default_rng(0)
    fake = {
        "x": rng.standard_normal((1, L, D), dtype=np.float32),
        "cond": rng.standard_normal((1, D), dtype=np.float32),
        "w_adaln1": rng.standard_normal((D, 3 * D), dtype=np.float32) * 0.02,
        "b_adaln1": np.zeros(3 * D, np.float32),
        "w_qkv": rng.standard_normal((D, 3 * D), dtype=np.float32) * D**-0.5,
        "b_qkv": np.zeros(3 * D, np.float32),
        "w_attn_out": rng.standard_normal((D, D), dtype=np.float32) * D**-0.5,
        "b_attn_out": np.zeros(D, np.float32),
        "w_adaln2": rng.standard_normal((D, 3 * D), dtype=np.float32) * 0.02,
        "b_adaln2": np.zeros(3 * D, np.float32),
        "w_ffn1": rng.standard_normal((D, DM), dtype=np.float32) * D**-0.5,
        "b_ffn1": np.zeros(DM, np.float32),
        "w_ffn2": rng.standard_normal((DM, D), dtype=np.float32) * DM**-0.5,
        "b_ffn2": np.zeros(D, np.float32),
    }
    out = kernel(**fake)
    print("out", out.shape, out.dtype, np.abs(out).max())
